# revision 12
# baseline (speedup 1.0000x reference)
"""Causal rotary self-attention Trainium2 kernel (8 NeuronCores).

Problem: B=4, N=1024, D=1024, H=16, DH=64.
  LayerNorm -> QKV proj -> RoPE(q,k) -> causal attention -> out proj.

Sharding: 8 cores = 4 batches x 2 head-halves (Megatron-style).  Each core
projects its 8 heads' q/k/v with its weight slice, runs attention for those
heads, and produces a partial output projection in bf16; the host sums the
two partials per batch.

Key design points (v2):
  - LayerNorm is computed entirely on the HOST (xn = (x-mu)*rstd*gamma+beta
    in fp32); the device receives the normalized transposed activations, so
    the device kernel is pure matmul + RoPE + attention.
  - All startup-critical input DMAs are serialized IN CONSUMPTION ORDER on a
    single queue (sync): perm, wqk(q0k0), cos, sinm, xT (dc-major), tri,
    remaining wqk, wv, wo.  Compute chases the DMA stream chunk-by-chunk.
  - A run of warm-up matmuls on a scratch tile covers the initial DMA window
    so the PE HAM clock-gate is released (K=8/8) before real work arrives.
  - scores S^T = kT.T @ qT per (head-pair, k-block, 512-chunk); the two heads
    of a pair run concurrently in disjoint 64-row PE groups, one ScalarE exp
    covers both heads' chunks.  Emission is software-pipelined at chunk
    granularity via a filler deque.
  - attn@V: lhsT = v3 with a ones column -> row 64 is the softmax
    denominator; the PSUM drain CAST carries the denominator row along
    (no separate ScalarE copy); reciprocal on DVE + gpsimd broadcast.
  - output projection: half0 (ic 0,1) accumulates early and drains to an
    SBUF staging tile; half1 accumulates ic 2,3 plus an identity-matmul of
    the staged half0 into the same PSUM group, so a single bf16 partial
    [N, D] is written out per core (host adds the 2 per-batch partials).
"""

import numpy as np
from collections import deque

B, N, D = 4, 1024, 1024
H, DH = 16, 64
EPS = 1e-5
P = 128
NHL = 8          # heads per core
FL = NHL * DH    # local features per core (512)
NWARM = 20      # PE warm-up matmuls at kernel start

_cache = {}


def _build_module(has_mask):
    import concourse.bass as bass
    import concourse.bacc as bacc
    import concourse.tile as tile
    import concourse.mybir as mybir
    from concourse.masks import make_identity

    f32 = mybir.dt.float32
    bf16 = mybir.dt.bfloat16
    AF = mybir.ActivationFunctionType
    OP = mybir.AluOpType

    nc = bacc.Bacc("TRN2", target_bir_lowering=False, debug=False, num_devices=8)

    NT = N // P    # 8 token chunks
    ND = D // P    # 8 contraction chunks
    NIC = FL // P  # 4 inner chunks
    NFC = 2 * FL // P  # 8 feature chunks (q/k interleaved: q0,k0,q1,k1,...)

    # wqk packed fc-major: [P, NFC, ND, P]
    xt_in = nc.dram_tensor("xt_in", [P, ND * N], bf16, kind="ExternalInput").ap()
    wqk_in = nc.dram_tensor("wqk_in", [P, NFC * ND * P], bf16, kind="ExternalInput").ap()
    wv_in = nc.dram_tensor("wv_in", [P, ND * FL], bf16, kind="ExternalInput").ap()
    wo_in = nc.dram_tensor("wo_in", [P, NIC * D], bf16, kind="ExternalInput").ap()
    cos_in = nc.dram_tensor("cos_in", [P, N], bf16, kind="ExternalInput").ap()
    sinm_in = nc.dram_tensor("sinm_in", [P, N], bf16, kind="ExternalInput").ap()
    tri_in = nc.dram_tensor("tri_in", [P, P], bf16, kind="ExternalInput").ap()
    perm_in = nc.dram_tensor("perm_in", [P, P], bf16, kind="ExternalInput").ap()
    if has_mask:
        madd_in = nc.dram_tensor("madd_in", [P, 8], f32, kind="ExternalInput").ap()
    out_p = nc.dram_tensor("out_p", [N, D], bf16, kind="ExternalOutput").ap()

    with tile.TileContext(nc) as tc:
        import contextlib
        ctx = contextlib.ExitStack()
        with ctx:
            consts = ctx.enter_context(tc.tile_pool(name="consts", bufs=1))
            big = ctx.enter_context(tc.tile_pool(name="big", bufs=1))
            tmp = ctx.enter_context(tc.tile_pool(name="tmp", bufs=3))
            pt_pool = ctx.enter_context(tc.tile_pool(name="pt_pool", bufs=3))
            avp = ctx.enter_context(tc.tile_pool(name="avp", bufs=3))
            small = ctx.enter_context(tc.tile_pool(name="small", bufs=2))
            bc_pool = ctx.enter_context(tc.tile_pool(name="bc_pool", bufs=3))
            out_pool = ctx.enter_context(tc.tile_pool(name="out_pool", bufs=2))
            psum = ctx.enter_context(tc.tile_pool(name="psum", bufs=2, space="PSUM"))

            # ---- warm-up scratch (device-built, no DMA dependency) ----
            ws = consts.tile([P, 512], bf16)
            nc.vector.memset(ws, 0.125)

            # ---- input DMAs: ONE ordered stream on the sync queue,
            # sequenced in consumption order (first MM needs wqkA + xT dc0)
            wqk_sb = consts.tile([P, NFC, ND, P], bf16)
            nc.sync.dma_start(out=wqk_sb[:, 0:2], in_=wqk_in[:, 0:2 * ND * P])
            xT_sb = big.tile([P, ND, N], bf16)
            nc.sync.dma_start(out=xT_sb[:, 0:2, :], in_=xt_in[:, 0:2 * N])
            cos_sb = consts.tile([P, N], bf16)
            nc.sync.dma_start(out=cos_sb, in_=cos_in)
            sinm_sb = consts.tile([P, N], bf16)
            nc.sync.dma_start(out=sinm_sb, in_=sinm_in)
            perm_sb = consts.tile([P, P], bf16)
            nc.sync.dma_start(out=perm_sb, in_=perm_in)
            for q4 in range(1, 4):
                nc.sync.dma_start(
                    out=xT_sb[:, 2 * q4:2 * q4 + 2, :],
                    in_=xt_in[:, 2 * q4 * N:(2 * q4 + 2) * N])
            tri_sb = consts.tile([P, P], bf16)
            nc.sync.dma_start(out=tri_sb, in_=tri_in)
            for j in range(1, 4):
                nc.sync.dma_start(
                    out=wqk_sb[:, 2 * j:2 * j + 2],
                    in_=wqk_in[:, 2 * j * ND * P:(2 * j + 2) * ND * P])
            wv_sb = consts.tile([P, ND, FL], bf16)
            nc.sync.dma_start(out=wv_sb, in_=wv_in)
            wo_sb = consts.tile([P, NIC, D], bf16)
            nc.sync.dma_start(out=wo_sb, in_=wo_in)
            if has_mask:
                madd_sb = consts.tile([P, 8], f32)
                nc.sync.dma_start(out=madd_sb, in_=madd_in)

            # ---- constants built on device ----
            idb = consts.tile([P, P], bf16)
            make_identity(nc, idb)

            # ---- PE warm-up: release the HAM clock-gate during the DMA
            # window.  Results are never read; banks recycle via the "s" tag
            # ring before the first real score tile needs them.
            for w in range(NWARM):
                wp = psum.tile([P, 2, 512], f32, tag="s", bufs=2, name=f"warm{w}")
                nc.tensor.matmul(wp[:, 0, :], lhsT=ws[:, 0:P], rhs=ws,
                                 start=True, stop=True, skip_group_check=True)

            # ================= work streams (emitted via filler deque) ======
            rope_sb = big.tile([P, NFC, N], bf16)
            v3 = big.tile([P, NT, NHL, DH + 1], bf16)
            nc.vector.memset(v3[:, :, :, DH:DH + 1], 1.0)
            aoT_sb = big.tile([P, NIC, N], bf16)
            o0_sb = big.tile([P, NT, N], bf16)

            F = deque()          # filler thunks; each emits ~1-2 matmuls
            marks = {}           # group name -> count emitted marker

            def drain(n):
                for _ in range(min(n, len(F))):
                    F.popleft()()

            def drain_group(g):
                while marks.get(g, 0) > 0:
                    F.popleft()()

            def addF(fn, group=None):
                if group is not None:
                    marks[group] = marks.get(group, 0) + 1

                    def wrapped():
                        fn()
                        marks[group] -= 1
                    F.append(wrapped)
                else:
                    F.append(fn)

            # ---- q/k projection + RoPE for one 128-feature chunk ----
            # fci indexes the interleaved order q0,k0,q1,k1,... ; rope_sb
            # rows use the same order.
            def emit_fc(fci, via=None):
                mms = [psum.tile([P, 512], f32, tag="mm", name=f"qk{fci}_{t}")
                       for t in range(2)]

                def mm_pair(dc):
                    for t in range(2):
                        nc.tensor.matmul(
                            mms[t], lhsT=wqk_sb[:, fci, dc, :],
                            rhs=xT_sb[:, dc, t * 512:(t + 1) * 512],
                            start=(dc == 0), stop=(dc == ND - 1))

                def rope(t):
                    cs = slice(t * 512, (t + 1) * 512)
                    t1 = tmp.tile([P, 512], bf16, name=f"t1_{fci}_{t}", tag="t1")
                    nc.vector.tensor_tensor(out=t1, in0=mms[t],
                                            in1=cos_sb[:, cs], op=OP.mult)
                    t2 = tmp.tile([P, 512], bf16, name=f"t2_{fci}_{t}", tag="t2")
                    nc.vector.tensor_tensor(out=t2, in0=mms[t],
                                            in1=sinm_sb[:, cs], op=OP.mult)
                    rot = psum.tile([P, 512], f32, tag="av", name=f"rot{fci}_{t}")
                    nc.tensor.matmul(rot, lhsT=perm_sb, rhs=t2,
                                     start=True, stop=True)
                    nc.vector.tensor_tensor(out=rope_sb[:, fci, cs], in0=rot,
                                            in1=t1, op=OP.add)

                steps = ([lambda dc=dc: mm_pair(dc) for dc in range(ND)]
                         + [lambda: rope(0), lambda: rope(1)])
                if via is None:
                    for s in steps:
                        s()
                else:
                    for s in steps:
                        addF(s, group=via)

            # ---- v projection for one 128-token chunk ----
            def emit_v(kc, via=None):
                mm = psum.tile([P, FL], f32, tag="mm", name=f"v{kc}")

                def vmm(dc):
                    nc.tensor.matmul(mm, lhsT=xT_sb[:, dc, kc * P:(kc + 1) * P],
                                     rhs=wv_sb[:, dc, :],
                                     start=(dc == 0), stop=(dc == ND - 1))

                def vdrain():
                    nc.scalar.copy(
                        out=v3[:, kc, :, 0:DH],
                        in_=mm.rearrange("p (h c) -> p h c", h=NHL))

                steps = ([lambda dc=dc: vmm(dc) for dc in range(ND)]
                         + [vdrain])
                for s in steps:
                    if via is None:
                        s()
                    else:
                        addF(s, group=via)

            # ---- scores + exp for a head pair, zipped with fillers ----
            all_pt = {}

            def zip_pair(m, fills_per_chunk=3):
                hs = (2 * m, 2 * m + 1)
                qTs = [rope_sb[(h % 2) * 64:(h % 2) * 64 + 64, 2 * m, :]
                       for h in hs]
                kTs = [rope_sb[(h % 2) * 64:(h % 2) * 64 + 64, 2 * m + 1, :]
                       for h in hs]
                pts = []
                for ki in range(NT):
                    q0 = ki * P
                    span = N - q0
                    pt = pt_pool.tile([P, 2, span], bf16, tag=f"pt{ki}",
                                      name=f"pt{m}_{ki}")
                    pts.append(pt)
                    for c in range((span + 511) // 512):
                        cw = min(512, span - c * 512)
                        sp = psum.tile([P, 2, 512], f32, tag="s", bufs=2,
                                       name=f"s{m}_{ki}_{c}")
                        for a in range(2):
                            diag = (c == 0)
                            nc.tensor.matmul(
                                sp[:, a, 0:cw],
                                lhsT=kTs[a][:, ki * P:(ki + 1) * P],
                                rhs=qTs[a][:, q0 + c * 512: q0 + c * 512 + cw],
                                start=True, stop=not diag)
                            if diag:
                                nc.tensor.matmul(
                                    sp[:, a, 0:P], lhsT=tri_sb, rhs=idb,
                                    start=False, stop=True,
                                    skip_group_check=True)
                        if has_mask:
                            nc.scalar.activation(
                                out=pt[:, :, c * 512:c * 512 + cw],
                                in_=sp[:, :, 0:cw], func=AF.Exp,
                                scale=float(DH) ** -0.5,
                                bias=madd_sb[:, ki:ki + 1])
                        else:
                            nc.scalar.activation(
                                out=pt[:, :, c * 512:c * 512 + cw],
                                in_=sp[:, :, 0:cw], func=AF.Exp,
                                scale=float(DH) ** -0.5)
                        drain(fills_per_chunk)
                for a, h in enumerate(hs):
                    all_pt[h] = (pts, a)

            # ---- attn@V + normalize per (head-pair, 512-token chunk) ----
            # The PSUM av tile is freed immediately by one DVE CAST that
            # carries rows 0..64 (values + denominator) into the pair-shared
            # SBUF tile av_pair[65, 1024]; recip + gpsimd broadcast + one DVE
            # multiply per head then normalize straight into aoT.
            av_state = {}

            def emit_av_A(h, cc, via=None):
                pts, a = all_pt[h]
                clo, chi = cc * 512, (cc + 1) * 512
                kis = [ki for ki in range(NT) if ki * P < chi]
                av = psum.tile([DH + 1, 512], f32, tag="av", name=f"av{h}_{cc}")
                pair_key = (h // 2 * 2, cc)
                if a == 0:
                    avs = avp.tile([DH, 1024], f32, name=f"avs{h}_{cc}",
                                   tag="avs")
                    den = small.tile([1, 1024], f32, name=f"den{h}_{cc}",
                                     tag="den")
                    av_state[pair_key] = (avs, den)
                else:
                    avs, den = av_state[pair_key]

                def avmm(idx, ki):
                    qlo = max(clo, ki * P)
                    nc.tensor.matmul(
                        av[:, qlo - clo:512],
                        lhsT=v3[:, ki, h, :],
                        rhs=pts[ki][:, a, qlo - ki * P:chi - ki * P],
                        start=(idx == 0), stop=(idx == len(kis) - 1))

                def drain_ps():
                    nc.vector.tensor_copy(
                        out=avs[:, a * 512:(a + 1) * 512], in_=av[0:DH, :])
                    # partition 64 -> 0 shift: ScalarE only (DVE is lane-locked)
                    nc.scalar.copy(out=den[0:1, a * 512:(a + 1) * 512],
                                   in_=av[DH:DH + 1, :])

                steps = ([lambda i=i, ki=ki: avmm(i, ki)
                          for i, ki in enumerate(kis)] + [drain_ps])
                for s in steps:
                    if via is None:
                        s()
                    else:
                        addF(s, group=via)

            def emit_av_B(h0, cc, via=None):
                avs, den = av_state[(h0, cc)]
                ic = h0 // 2
                clo, chi = cc * 512, (cc + 1) * 512

                def norm():
                    rr = small.tile([1, 1024], f32, name=f"rr{h0}_{cc}",
                                    tag="rr")
                    nc.vector.reciprocal_approx_fast(out=rr, in_=den)
                    for a in range(2):
                        bc = bc_pool.tile([DH, 512], f32, name=f"bc{h0}_{cc}_{a}",
                                          tag="bc", bufs=3)
                        nc.gpsimd.partition_broadcast(
                            bc, rr[0:1, a * 512:(a + 1) * 512])
                        nc.vector.tensor_tensor(
                            out=aoT_sb[a * DH:(a + 1) * DH, ic, clo:chi],
                            in0=avs[0:DH, a * 512:(a + 1) * 512],
                            in1=bc, op=OP.mult)

                if via is None:
                    norm()
                else:
                    addF(norm, group=via)

            def emit_av_pair(h0, h1, via=None, ccs=(0, 1)):
                for cc in ccs:
                    emit_av_A(h0, cc, via=via)
                    emit_av_A(h1, cc, via=via)
                    emit_av_B(h0, cc, via=via)

            # ---- output projection ----
            # half 0: accumulate ic 0,1 -> stage to o0_sb (bf16).
            # half 1: accumulate ic 2,3 + idb@o0_sb -> single bf16 output.
            def emit_outproj(half, via=None, tcis=None, alt_ring=False):
                def opmm(ic, n2, tci, mms, last):
                    nc.tensor.matmul(
                        mms[n2], lhsT=aoT_sb[:, ic, tci * P:(tci + 1) * P],
                        rhs=wo_sb[:, ic, n2 * 512:(n2 + 1) * 512],
                        start=(ic == 2 * half), stop=last)

                def opadd0(n2, tci, mms):
                    nc.tensor.matmul(
                        mms[n2], lhsT=idb,
                        rhs=o0_sb[:, tci, n2 * 512:(n2 + 1) * 512],
                        start=False, stop=True)

                def opout(tci, mms):
                    if half == 0:
                        nc.vector.tensor_copy(
                            out=o0_sb[:, tci, 0:512], in_=mms[0])
                        nc.scalar.copy(
                            out=o0_sb[:, tci, 512:1024], in_=mms[1])
                    else:
                        ot = out_pool.tile([P, N], bf16,
                                           name=f"ot{tci}", tag="ot")
                        nc.vector.tensor_copy(out=ot[:, 0:512], in_=mms[0])
                        nc.scalar.copy(out=ot[:, 512:1024], in_=mms[1])
                        nc.scalar.dma_start(
                            out=out_p[tci * P:(tci + 1) * P, :], in_=ot)

                for tci in (range(NT) if tcis is None else tcis):
                    tg = "s" if (alt_ring and tci % 2) else "mm"
                    mms = [psum.tile([P, 512], f32, tag=tg,
                                     name=f"op{half}_{tci}_{n2}")
                           for n2 in range(2)]
                    steps = []
                    for n2 in range(2):
                        for ic in (2 * half, 2 * half + 1):
                            last = (half == 0 and ic == 1)
                            steps.append(
                                lambda ic=ic, n2=n2, t=tci, mm=mms, l=last:
                                opmm(ic, n2, t, mm, l))
                        if half == 1:
                            steps.append(
                                lambda n2=n2, t=tci, mm=mms: opadd0(n2, t, mm))
                    steps.append(lambda t=tci, mm=mms: opout(t, mm))
                    for s in steps:
                        if via is None:
                            s()
                        else:
                            addF(s, group=via)

            # ================= emission schedule ============================
            # NOTE: deferred (via=) steps are strictly FIFO; a consumer may
            # only be emitted (direct or deferred) after every producer it
            # needs is ahead of it in the deque or already drained.
            emit_fc(0)                # q0 (DMA-paced)
            emit_fc(1)                # k0
            emit_fc(2, via="fcB")     # q1
            emit_fc(3, via="fcB")     # k1
            zip_pair(0, fills_per_chunk=3)
            drain_group("fcB")
            emit_fc(4, via="fcC")     # q2
            emit_fc(5, via="fcC")     # k2
            for kc in range(NT):
                emit_v(kc, via="v")
            zip_pair(1, fills_per_chunk=5)
            drain_group("fcC")
            emit_av_pair(0, 1, via="w1")   # sits after v in the deque
            emit_fc(6, via="fcD")     # q3
            emit_fc(7, via="fcD")     # k3
            zip_pair(2, fills_per_chunk=6)
            drain_group("fcD")        # flushes v/w1 remnants first (FIFO)
            emit_av_pair(2, 3, via="w2")
            emit_outproj(0, via="op0")
            emit_av_pair(4, 5, via="p45")
            zip_pair(3, fills_per_chunk=4)
            while F:
                F.popleft()()
            emit_av_pair(6, 7, ccs=(0,))
            emit_av_pair(6, 7, ccs=(1,))
            emit_outproj(1, tcis=range(0, 4), alt_ring=True)
            emit_outproj(1, tcis=range(4, 8), alt_ring=True)

    nc.compile()
    return nc


def kernel(x, rotary_time_emb, x_mask, ln_gamma, ln_beta, w_qkv, w_out, b_out):
    import ml_dtypes
    from concourse import bass_utils

    bf = ml_dtypes.bfloat16
    x = np.asarray(x, np.float32)
    emb = np.asarray(rotary_time_emb, np.float32)
    x_mask = np.asarray(x_mask)
    ln_gamma = np.asarray(ln_gamma, np.float32)
    ln_beta = np.asarray(ln_beta, np.float32)
    w_qkv = np.asarray(w_qkv, np.float32)
    w_out = np.asarray(w_out, np.float32)
    b_out = np.asarray(b_out, np.float32)

    has_mask = bool(np.any(~x_mask.astype(bool)))

    if has_mask not in _cache:
        _cache[has_mask] = _build_module(has_mask)
    nc = _cache[has_mask]

    inner = H * DH
    wq, wk, wv = w_qkv[0:inner], w_qkv[inner:2 * inner], w_qkv[2 * inner:]

    # Host-side LayerNorm (fp32, matches the reference exactly)
    mu = x.mean(-1, keepdims=True)
    var = ((x - mu) ** 2).mean(-1, keepdims=True)
    xn = (x - mu) / np.sqrt(var + EPS) * ln_gamma + ln_beta   # (B, N, D)

    cos = np.cos(emb)                       # (B, N, DH)
    sin = np.sin(emb)

    # block-swap permutation for rotate_half in the transposed layout
    perm = np.zeros((P, P), np.float32)
    o = np.arange(P)
    src = np.where((o % 64) < 32, o + 32, o - 32)
    perm[o, src] = 1.0

    def pack(a):   # [K*P, F] -> [P, K*F] with K-chunks per partition
        kp, f = a.shape
        return np.ascontiguousarray(
            a.reshape(kp // P, P, f).transpose(1, 0, 2).reshape(P, -1)
            .astype(bf))

    in_maps = []
    for core in range(8):
        b, hh = core // 2, core % 2
        sl = slice(hh * FL, (hh + 1) * FL)
        # fc-major q/k interleave: q0,k0,q1,k1,...  each [P, ND, P]
        wq_c, wk_c = wq[sl], wk[sl]          # (FL, D)
        fcs = []
        for i in range(FL // P):
            for w_half in (wq_c, wk_c):
                blk = w_half[i * P:(i + 1) * P]          # (P, D)
                fcs.append(pack(np.ascontiguousarray(blk.T)))  # [P, ND*P]
        wqk_packed = np.ascontiguousarray(
            np.stack(fcs, 1).reshape(P, -1))             # [P, NFC*ND*P]

        m = {
            "xt_in": pack(np.ascontiguousarray(xn[b].T)),
            "wqk_in": wqk_packed,
            "wv_in": pack(np.ascontiguousarray(wv[sl].T)),
            "wo_in": pack(np.ascontiguousarray(w_out[:, sl].T)),
            "perm_in": np.ascontiguousarray(perm.astype(bf)),
        }
        cT = cos[b].T                        # (DH, N)
        sT = sin[b].T
        cos2 = np.concatenate([cT, cT], 0)   # (128, N)
        sinm = np.concatenate([sT[32:64], -sT[0:32], sT[32:64], -sT[0:32]], 0)
        m["cos_in"] = np.ascontiguousarray(cos2.astype(bf))
        m["sinm_in"] = np.ascontiguousarray(sinm.astype(bf))
        k_idx = np.arange(P)[:, None]
        q_idx = np.arange(P)[None, :]
        trimask = np.where(k_idx <= q_idx, 0.0, -30000.0)   # [k, q]
        m["tri_in"] = np.ascontiguousarray(trimask.T.astype(bf))
        if has_mask:
            madd = np.where(x_mask[b].astype(bool), 0.0, -30000.0)
            m["madd_in"] = np.ascontiguousarray(
                madd.reshape(8, P).T.astype(np.float32))   # [p, kc]
        in_maps.append(m)

    res = bass_utils.run_bass_kernel_spmd(nc, in_maps, core_ids=list(range(8)))

    out = np.empty((B, N, D), np.float32)
    for b in range(B):
        out[b] = (res.results[2 * b]["out_p"].astype(np.float32)
                  + res.results[2 * b + 1]["out_p"].astype(np.float32))
    out += b_out[None, None, :]
    return out


# revision 13
# speedup vs baseline: 1.0220x; 1.0220x over previous
"""Causal rotary self-attention Trainium2 kernel (8 NeuronCores).

Problem: B=4, N=1024, D=1024, H=16, DH=64.
  LayerNorm -> QKV proj -> RoPE(q,k) -> causal attention -> out proj.

Sharding: 8 cores = 4 batches x 2 head-halves (Megatron-style).  Each core
projects its 8 heads' q/k/v with its weight slice, runs attention for those
heads, and produces a partial output projection in bf16; the host sums the
two partials per batch.

Key design points (v2):
  - LayerNorm is computed entirely on the HOST (xn = (x-mu)*rstd*gamma+beta
    in fp32); the device receives the normalized transposed activations, so
    the device kernel is pure matmul + RoPE + attention.
  - All startup-critical input DMAs are serialized IN CONSUMPTION ORDER on a
    single queue (sync): perm, wqk(q0k0), cos, sinm, xT (dc-major), tri,
    remaining wqk, wv, wo.  Compute chases the DMA stream chunk-by-chunk.
  - A run of warm-up matmuls on a scratch tile covers the initial DMA window
    so the PE HAM clock-gate is released (K=8/8) before real work arrives.
  - scores S^T = kT.T @ qT per (head-pair, k-block, 512-chunk); the two heads
    of a pair run concurrently in disjoint 64-row PE groups, one ScalarE exp
    covers both heads' chunks.  Emission is software-pipelined at chunk
    granularity via a filler deque.
  - attn@V: lhsT = v3 with a ones column -> row 64 is the softmax
    denominator; the PSUM drain CAST carries the denominator row along
    (no separate ScalarE copy); reciprocal on DVE + gpsimd broadcast.
  - output projection: half0 (ic 0,1) accumulates early and drains to an
    SBUF staging tile; half1 accumulates ic 2,3 plus an identity-matmul of
    the staged half0 into the same PSUM group, so a single bf16 partial
    [N, D] is written out per core (host adds the 2 per-batch partials).
"""

import numpy as np
from collections import deque

B, N, D = 4, 1024, 1024
H, DH = 16, 64
EPS = 1e-5
P = 128
NHL = 8          # heads per core
FL = NHL * DH    # local features per core (512)
NWARM = 20      # PE warm-up matmuls at kernel start

_cache = {}


def _build_module(has_mask):
    import concourse.bass as bass
    import concourse.bacc as bacc
    import concourse.tile as tile
    import concourse.mybir as mybir
    from concourse.masks import make_identity

    f32 = mybir.dt.float32
    bf16 = mybir.dt.bfloat16
    AF = mybir.ActivationFunctionType
    OP = mybir.AluOpType

    nc = bacc.Bacc("TRN2", target_bir_lowering=False, debug=False, num_devices=8)

    NT = N // P    # 8 token chunks
    ND = D // P    # 8 contraction chunks
    NIC = FL // P  # 4 inner chunks
    NFC = 2 * FL // P  # 8 feature chunks (q/k interleaved: q0,k0,q1,k1,...)

    # wqk packed fc-major: [P, NFC, ND, P]
    xt_in = nc.dram_tensor("xt_in", [P, ND * N], bf16, kind="ExternalInput").ap()
    wqk_in = nc.dram_tensor("wqk_in", [P, NFC * ND * P], bf16, kind="ExternalInput").ap()
    wv_in = nc.dram_tensor("wv_in", [P, ND * FL], bf16, kind="ExternalInput").ap()
    wo_in = nc.dram_tensor("wo_in", [P, NIC * D], bf16, kind="ExternalInput").ap()
    cos_in = nc.dram_tensor("cos_in", [P, N], bf16, kind="ExternalInput").ap()
    sinm_in = nc.dram_tensor("sinm_in", [P, N], bf16, kind="ExternalInput").ap()
    tri_in = nc.dram_tensor("tri_in", [P, P], bf16, kind="ExternalInput").ap()
    perm_in = nc.dram_tensor("perm_in", [P, P], bf16, kind="ExternalInput").ap()
    if has_mask:
        madd_in = nc.dram_tensor("madd_in", [P, 8], f32, kind="ExternalInput").ap()
    out_p = nc.dram_tensor("out_p", [N, D], bf16, kind="ExternalOutput").ap()

    with tile.TileContext(nc) as tc:
        import contextlib
        ctx = contextlib.ExitStack()
        with ctx:
            consts = ctx.enter_context(tc.tile_pool(name="consts", bufs=1))
            big = ctx.enter_context(tc.tile_pool(name="big", bufs=1))
            tmp = ctx.enter_context(tc.tile_pool(name="tmp", bufs=3))
            pt_pool = ctx.enter_context(tc.tile_pool(name="pt_pool", bufs=3))
            avp = ctx.enter_context(tc.tile_pool(name="avp", bufs=3))
            small = ctx.enter_context(tc.tile_pool(name="small", bufs=2))
            bc_pool = ctx.enter_context(tc.tile_pool(name="bc_pool", bufs=3))
            out_pool = ctx.enter_context(tc.tile_pool(name="out_pool", bufs=2))
            psum = ctx.enter_context(tc.tile_pool(name="psum", bufs=2, space="PSUM"))

            # ---- warm-up scratch (device-built, no DMA dependency) ----
            ws = consts.tile([P, 512], bf16)
            nc.vector.memset(ws, 0.125)

            # ---- input DMAs: ONE ordered stream on the sync queue,
            # sequenced in consumption order (first MM needs wqkA + xT dc0)
            wqk_sb = consts.tile([P, NFC, ND, P], bf16)
            nc.sync.dma_start(out=wqk_sb[:, 0:2], in_=wqk_in[:, 0:2 * ND * P])
            xT_sb = big.tile([P, ND, N], bf16)
            nc.sync.dma_start(out=xT_sb[:, :, 0:512],
                              in_=xt_in[:, 0:ND * 512])
            cos_sb = consts.tile([P, N], bf16)
            nc.sync.dma_start(out=cos_sb, in_=cos_in)
            sinm_sb = consts.tile([P, N], bf16)
            nc.sync.dma_start(out=sinm_sb, in_=sinm_in)
            perm_sb = consts.tile([P, P], bf16)
            nc.sync.dma_start(out=perm_sb, in_=perm_in)
            nc.sync.dma_start(out=xT_sb[:, :, 512:1024],
                              in_=xt_in[:, ND * 512:2 * ND * 512])
            tri_sb = consts.tile([P, P], bf16)
            nc.sync.dma_start(out=tri_sb, in_=tri_in)
            for j in range(1, 4):
                nc.sync.dma_start(
                    out=wqk_sb[:, 2 * j:2 * j + 2],
                    in_=wqk_in[:, 2 * j * ND * P:(2 * j + 2) * ND * P])
            wv_sb = consts.tile([P, ND, FL], bf16)
            nc.sync.dma_start(out=wv_sb, in_=wv_in)
            wo_sb = consts.tile([P, NIC, D], bf16)
            nc.sync.dma_start(out=wo_sb, in_=wo_in)
            if has_mask:
                madd_sb = consts.tile([P, 8], f32)
                nc.sync.dma_start(out=madd_sb, in_=madd_in)

            # ---- constants built on device ----
            idb = consts.tile([P, P], bf16)
            make_identity(nc, idb)

            # ---- PE warm-up: release the HAM clock-gate during the DMA
            # window.  Results are never read; banks recycle via the "s" tag
            # ring before the first real score tile needs them.
            for w in range(NWARM):
                wp = psum.tile([P, 2, 512], f32, tag="s", bufs=2, name=f"warm{w}")
                nc.tensor.matmul(wp[:, 0, :], lhsT=ws[:, 0:P], rhs=ws,
                                 start=True, stop=True, skip_group_check=True)

            # ================= work streams (emitted via filler deque) ======
            rope_sb = big.tile([P, NFC, N], bf16)
            v3 = big.tile([P, NT, NHL, DH + 1], bf16)
            nc.vector.memset(v3[:, :, :, DH:DH + 1], 1.0)
            aoT_sb = big.tile([P, NIC, N], bf16)
            o0_sb = big.tile([P, NT, N], bf16)

            F = deque()          # filler thunks; each emits ~1-2 matmuls
            marks = {}           # group name -> count emitted marker

            def drain(n):
                for _ in range(min(n, len(F))):
                    F.popleft()()

            def drain_group(g):
                while marks.get(g, 0) > 0:
                    F.popleft()()

            def addF(fn, group=None):
                if group is not None:
                    marks[group] = marks.get(group, 0) + 1

                    def wrapped():
                        fn()
                        marks[group] -= 1
                    F.append(wrapped)
                else:
                    F.append(fn)

            # ---- q/k projection + RoPE for one 128-feature chunk ----
            # fci indexes the interleaved order q0,k0,q1,k1,... ; rope_sb
            # rows use the same order.
            def emit_fc(fci, via=None):
                mms = [psum.tile([P, 512], f32, tag="mm", name=f"qk{fci}_{t}")
                       for t in range(2)]

                def mm_pair(dc):
                    for t in range(2):
                        nc.tensor.matmul(
                            mms[t], lhsT=wqk_sb[:, fci, dc, :],
                            rhs=xT_sb[:, dc, t * 512:(t + 1) * 512],
                            start=(dc == 0), stop=(dc == ND - 1))

                def rope(t):
                    cs = slice(t * 512, (t + 1) * 512)
                    t1 = tmp.tile([P, 512], bf16, name=f"t1_{fci}_{t}", tag="t1")
                    nc.vector.tensor_tensor(out=t1, in0=mms[t],
                                            in1=cos_sb[:, cs], op=OP.mult)
                    t2 = tmp.tile([P, 512], bf16, name=f"t2_{fci}_{t}", tag="t2")
                    nc.vector.tensor_tensor(out=t2, in0=mms[t],
                                            in1=sinm_sb[:, cs], op=OP.mult)
                    rot = psum.tile([P, 512], f32, tag="av", name=f"rot{fci}_{t}")
                    nc.tensor.matmul(rot, lhsT=perm_sb, rhs=t2,
                                     start=True, stop=True)
                    nc.vector.tensor_tensor(out=rope_sb[:, fci, cs], in0=rot,
                                            in1=t1, op=OP.add)

                def mm_one(t, dc):
                    nc.tensor.matmul(
                        mms[t], lhsT=wqk_sb[:, fci, dc, :],
                        rhs=xT_sb[:, dc, t * 512:(t + 1) * 512],
                        start=(dc == 0), stop=(dc == ND - 1))

                steps = ([lambda dc=dc: mm_one(0, dc) for dc in range(ND)]
                         + [lambda: rope(0)]
                         + [lambda dc=dc: mm_one(1, dc) for dc in range(ND)]
                         + [lambda: rope(1)])
                if via is None:
                    for s in steps:
                        s()
                else:
                    for s in steps:
                        addF(s, group=via)

            # ---- v projection for one 128-token chunk ----
            def emit_v(kc, via=None):
                mm = psum.tile([P, FL], f32, tag="mm", name=f"v{kc}")

                def vmm(dc):
                    nc.tensor.matmul(mm, lhsT=xT_sb[:, dc, kc * P:(kc + 1) * P],
                                     rhs=wv_sb[:, dc, :],
                                     start=(dc == 0), stop=(dc == ND - 1))

                def vdrain():
                    nc.scalar.copy(
                        out=v3[:, kc, :, 0:DH],
                        in_=mm.rearrange("p (h c) -> p h c", h=NHL))

                steps = ([lambda dc=dc: vmm(dc) for dc in range(ND)]
                         + [vdrain])
                for s in steps:
                    if via is None:
                        s()
                    else:
                        addF(s, group=via)

            # ---- scores + exp for a head pair, zipped with fillers ----
            all_pt = {}

            def zip_pair(m, fills_per_chunk=3):
                hs = (2 * m, 2 * m + 1)
                qTs = [rope_sb[(h % 2) * 64:(h % 2) * 64 + 64, 2 * m, :]
                       for h in hs]
                kTs = [rope_sb[(h % 2) * 64:(h % 2) * 64 + 64, 2 * m + 1, :]
                       for h in hs]
                pts = []
                for ki in range(NT):
                    q0 = ki * P
                    span = N - q0
                    pt = pt_pool.tile([P, 2, span], bf16, tag=f"pt{ki}",
                                      name=f"pt{m}_{ki}")
                    pts.append(pt)
                    for c in range((span + 511) // 512):
                        cw = min(512, span - c * 512)
                        sp = psum.tile([P, 2, 512], f32, tag="s", bufs=2,
                                       name=f"s{m}_{ki}_{c}")
                        for a in range(2):
                            diag = (c == 0)
                            nc.tensor.matmul(
                                sp[:, a, 0:cw],
                                lhsT=kTs[a][:, ki * P:(ki + 1) * P],
                                rhs=qTs[a][:, q0 + c * 512: q0 + c * 512 + cw],
                                start=True, stop=not diag)
                            if diag:
                                nc.tensor.matmul(
                                    sp[:, a, 0:P], lhsT=tri_sb, rhs=idb,
                                    start=False, stop=True,
                                    skip_group_check=True)
                        if has_mask:
                            nc.scalar.activation(
                                out=pt[:, :, c * 512:c * 512 + cw],
                                in_=sp[:, :, 0:cw], func=AF.Exp,
                                scale=float(DH) ** -0.5,
                                bias=madd_sb[:, ki:ki + 1])
                        else:
                            nc.scalar.activation(
                                out=pt[:, :, c * 512:c * 512 + cw],
                                in_=sp[:, :, 0:cw], func=AF.Exp,
                                scale=float(DH) ** -0.5)
                        drain(fills_per_chunk)
                for a, h in enumerate(hs):
                    all_pt[h] = (pts, a)

            # ---- attn@V + normalize per (head-pair, 512-token chunk) ----
            # The PSUM av tile is freed immediately by one DVE CAST that
            # carries rows 0..64 (values + denominator) into the pair-shared
            # SBUF tile av_pair[65, 1024]; recip + gpsimd broadcast + one DVE
            # multiply per head then normalize straight into aoT.
            av_state = {}

            def emit_av_A(h, cc, via=None):
                pts, a = all_pt[h]
                clo, chi = cc * 512, (cc + 1) * 512
                kis = [ki for ki in range(NT) if ki * P < chi]
                av = psum.tile([DH + 1, 512], f32, tag="av", name=f"av{h}_{cc}")
                pair_key = (h // 2 * 2, cc)
                if a == 0:
                    avs = avp.tile([DH, 1024], f32, name=f"avs{h}_{cc}",
                                   tag="avs")
                    den = small.tile([1, 1024], f32, name=f"den{h}_{cc}",
                                     tag="den")
                    av_state[pair_key] = (avs, den)
                else:
                    avs, den = av_state[pair_key]

                def avmm(idx, ki):
                    qlo = max(clo, ki * P)
                    nc.tensor.matmul(
                        av[:, qlo - clo:512],
                        lhsT=v3[:, ki, h, :],
                        rhs=pts[ki][:, a, qlo - ki * P:chi - ki * P],
                        start=(idx == 0), stop=(idx == len(kis) - 1))

                def drain_ps():
                    eng = nc.scalar if h >= 6 else nc.vector
                    if h >= 6:
                        eng.copy(out=avs[:, a * 512:(a + 1) * 512],
                                 in_=av[0:DH, :])
                    else:
                        eng.tensor_copy(out=avs[:, a * 512:(a + 1) * 512],
                                        in_=av[0:DH, :])
                    # partition 64 -> 0 shift: ScalarE only (DVE is lane-locked)
                    nc.scalar.copy(out=den[0:1, a * 512:(a + 1) * 512],
                                   in_=av[DH:DH + 1, :])

                steps = ([lambda i=i, ki=ki: avmm(i, ki)
                          for i, ki in enumerate(kis)] + [drain_ps])
                for s in steps:
                    if via is None:
                        s()
                    else:
                        addF(s, group=via)

            def emit_av_B(h0, cc, via=None):
                avs, den = av_state[(h0, cc)]
                ic = h0 // 2
                clo, chi = cc * 512, (cc + 1) * 512

                def norm():
                    rr = small.tile([1, 1024], f32, name=f"rr{h0}_{cc}",
                                    tag="rr")
                    nc.vector.reciprocal_approx_fast(out=rr, in_=den)
                    for a in range(2):
                        bc = bc_pool.tile([DH, 512], f32, name=f"bc{h0}_{cc}_{a}",
                                          tag="bc", bufs=3)
                        nc.gpsimd.partition_broadcast(
                            bc, rr[0:1, a * 512:(a + 1) * 512])
                        nc.vector.tensor_tensor(
                            out=aoT_sb[a * DH:(a + 1) * DH, ic, clo:chi],
                            in0=avs[0:DH, a * 512:(a + 1) * 512],
                            in1=bc, op=OP.mult)

                if via is None:
                    norm()
                else:
                    addF(norm, group=via)

            def emit_av_pair(h0, h1, via=None, ccs=(0, 1)):
                for cc in ccs:
                    emit_av_A(h0, cc, via=via)
                    emit_av_A(h1, cc, via=via)
                    emit_av_B(h0, cc, via=via)

            # ---- output projection ----
            # half 0: accumulate ic 0,1 -> stage to o0_sb (bf16).
            # half 1: accumulate ic 2,3 + idb@o0_sb -> single bf16 output.
            def emit_outproj(half, via=None, tcis=None, alt_ring=False):
                def opmm(ic, n2, tci, mms, last):
                    nc.tensor.matmul(
                        mms[n2], lhsT=aoT_sb[:, ic, tci * P:(tci + 1) * P],
                        rhs=wo_sb[:, ic, n2 * 512:(n2 + 1) * 512],
                        start=(ic == 2 * half), stop=last)

                def opadd0(n2, tci, mms):
                    nc.tensor.matmul(
                        mms[n2], lhsT=idb,
                        rhs=o0_sb[:, tci, n2 * 512:(n2 + 1) * 512],
                        start=False, stop=True)

                def opout(tci, mms):
                    if half == 0:
                        nc.vector.tensor_copy(
                            out=o0_sb[:, tci, 0:512], in_=mms[0])
                        nc.scalar.copy(
                            out=o0_sb[:, tci, 512:1024], in_=mms[1])
                    else:
                        ot = out_pool.tile([P, N], bf16,
                                           name=f"ot{tci}", tag="ot")
                        nc.vector.tensor_copy(out=ot[:, 0:512], in_=mms[0])
                        nc.scalar.copy(out=ot[:, 512:1024], in_=mms[1])
                        nc.scalar.dma_start(
                            out=out_p[tci * P:(tci + 1) * P, :], in_=ot)

                for tci in (range(NT) if tcis is None else tcis):
                    tg = "s" if (alt_ring and tci % 2) else "mm"
                    mms = [psum.tile([P, 512], f32, tag=tg,
                                     name=f"op{half}_{tci}_{n2}")
                           for n2 in range(2)]
                    steps = []
                    for n2 in range(2):
                        for ic in (2 * half, 2 * half + 1):
                            last = (half == 0 and ic == 1)
                            steps.append(
                                lambda ic=ic, n2=n2, t=tci, mm=mms, l=last:
                                opmm(ic, n2, t, mm, l))
                        if half == 1:
                            steps.append(
                                lambda n2=n2, t=tci, mm=mms: opadd0(n2, t, mm))
                    steps.append(lambda t=tci, mm=mms: opout(t, mm))
                    for s in steps:
                        if via is None:
                            s()
                        else:
                            addF(s, group=via)

            # ================= emission schedule ============================
            # NOTE: deferred (via=) steps are strictly FIFO; a consumer may
            # only be emitted (direct or deferred) after every producer it
            # needs is ahead of it in the deque or already drained.
            emit_fc(0)                # q0 (DMA-paced)
            emit_fc(1)                # k0
            emit_fc(2, via="fcB")     # q1
            emit_fc(3, via="fcB")     # k1
            zip_pair(0, fills_per_chunk=3)
            drain_group("fcB")
            emit_fc(4, via="fcC")     # q2
            emit_fc(5, via="fcC")     # k2
            for kc in range(NT):
                emit_v(kc, via="v")
            zip_pair(1, fills_per_chunk=5)
            drain_group("fcC")
            emit_av_pair(0, 1, via="w1")   # sits after v in the deque
            emit_fc(6, via="fcD")     # q3
            emit_fc(7, via="fcD")     # k3
            zip_pair(2, fills_per_chunk=6)
            drain_group("fcD")        # flushes v/w1 remnants first (FIFO)
            emit_av_pair(2, 3, via="w2")
            emit_outproj(0, via="op0")
            emit_av_pair(4, 5, via="p45")
            zip_pair(3, fills_per_chunk=4)
            while F:
                F.popleft()()
            emit_av_pair(6, 7, ccs=(0,))
            emit_av_A(6, 1)
            emit_av_A(7, 1)
            emit_outproj(1, tcis=range(0, 2), alt_ring=True)
            emit_av_B(6, 1)
            emit_outproj(1, tcis=range(2, 8), alt_ring=True)

    nc.compile()
    return nc


def kernel(x, rotary_time_emb, x_mask, ln_gamma, ln_beta, w_qkv, w_out, b_out):
    import ml_dtypes
    from concourse import bass_utils

    bf = ml_dtypes.bfloat16
    x = np.asarray(x, np.float32)
    emb = np.asarray(rotary_time_emb, np.float32)
    x_mask = np.asarray(x_mask)
    ln_gamma = np.asarray(ln_gamma, np.float32)
    ln_beta = np.asarray(ln_beta, np.float32)
    w_qkv = np.asarray(w_qkv, np.float32)
    w_out = np.asarray(w_out, np.float32)
    b_out = np.asarray(b_out, np.float32)

    has_mask = bool(np.any(~x_mask.astype(bool)))

    if has_mask not in _cache:
        _cache[has_mask] = _build_module(has_mask)
    nc = _cache[has_mask]

    inner = H * DH
    wq, wk, wv = w_qkv[0:inner], w_qkv[inner:2 * inner], w_qkv[2 * inner:]

    # Host-side LayerNorm (fp32, matches the reference exactly)
    mu = x.mean(-1, keepdims=True)
    var = ((x - mu) ** 2).mean(-1, keepdims=True)
    xn = (x - mu) / np.sqrt(var + EPS) * ln_gamma + ln_beta   # (B, N, D)

    cos = np.cos(emb)                       # (B, N, DH)
    sin = np.sin(emb)

    # block-swap permutation for rotate_half in the transposed layout
    perm = np.zeros((P, P), np.float32)
    o = np.arange(P)
    src = np.where((o % 64) < 32, o + 32, o - 32)
    perm[o, src] = 1.0

    def pack(a):   # [K*P, F] -> [P, K*F] with K-chunks per partition
        kp, f = a.shape
        return np.ascontiguousarray(
            a.reshape(kp // P, P, f).transpose(1, 0, 2).reshape(P, -1)
            .astype(bf))

    in_maps = []
    for core in range(8):
        b, hh = core // 2, core % 2
        sl = slice(hh * FL, (hh + 1) * FL)
        # fc-major q/k interleave: q0,k0,q1,k1,...  each [P, ND, P]
        wq_c, wk_c = wq[sl], wk[sl]          # (FL, D)
        fcs = []
        for i in range(FL // P):
            for w_half in (wq_c, wk_c):
                blk = w_half[i * P:(i + 1) * P]          # (P, D)
                fcs.append(pack(np.ascontiguousarray(blk.T)))  # [P, ND*P]
        wqk_packed = np.ascontiguousarray(
            np.stack(fcs, 1).reshape(P, -1))             # [P, NFC*ND*P]

        xnT = np.ascontiguousarray(xn[b].T)          # (D, N)
        xt_halves = [pack(np.ascontiguousarray(xnT[:, h * 512:(h + 1) * 512]))
                     for h in range(2)]               # each [P, ND*512]
        m = {
            "xt_in": np.ascontiguousarray(np.concatenate(xt_halves, 1)),
            "wqk_in": wqk_packed,
            "wv_in": pack(np.ascontiguousarray(wv[sl].T)),
            "wo_in": pack(np.ascontiguousarray(w_out[:, sl].T)),
            "perm_in": np.ascontiguousarray(perm.astype(bf)),
        }
        cT = cos[b].T                        # (DH, N)
        sT = sin[b].T
        cos2 = np.concatenate([cT, cT], 0)   # (128, N)
        sinm = np.concatenate([sT[32:64], -sT[0:32], sT[32:64], -sT[0:32]], 0)
        m["cos_in"] = np.ascontiguousarray(cos2.astype(bf))
        m["sinm_in"] = np.ascontiguousarray(sinm.astype(bf))
        k_idx = np.arange(P)[:, None]
        q_idx = np.arange(P)[None, :]
        trimask = np.where(k_idx <= q_idx, 0.0, -30000.0)   # [k, q]
        m["tri_in"] = np.ascontiguousarray(trimask.T.astype(bf))
        if has_mask:
            madd = np.where(x_mask[b].astype(bool), 0.0, -30000.0)
            m["madd_in"] = np.ascontiguousarray(
                madd.reshape(8, P).T.astype(np.float32))   # [p, kc]
        in_maps.append(m)

    res = bass_utils.run_bass_kernel_spmd(nc, in_maps, core_ids=list(range(8)))

    out = np.empty((B, N, D), np.float32)
    for b in range(B):
        out[b] = (res.results[2 * b]["out_p"].astype(np.float32)
                  + res.results[2 * b + 1]["out_p"].astype(np.float32))
    out += b_out[None, None, :]
    return out


# revision 14
# speedup vs baseline: 1.0570x; 1.0343x over previous
"""Causal rotary self-attention Trainium2 kernel (8 NeuronCores).

Problem: B=4, N=1024, D=1024, H=16, DH=64.
  LayerNorm -> QKV proj -> RoPE(q,k) -> causal attention -> out proj.

Sharding: 8 cores = 4 batches x 2 head-halves (Megatron-style).  Each core
projects its 8 heads' q/k/v with its weight slice, runs attention for those
heads, and produces a partial output projection in bf16; the host sums the
two partials per batch.

Key design points (v2):
  - LayerNorm is computed entirely on the HOST (xn = (x-mu)*rstd*gamma+beta
    in fp32); the device receives the normalized transposed activations, so
    the device kernel is pure matmul + RoPE + attention.
  - All startup-critical input DMAs are serialized IN CONSUMPTION ORDER on a
    single queue (sync): perm, wqk(q0k0), cos, sinm, xT (dc-major), tri,
    remaining wqk, wv, wo.  Compute chases the DMA stream chunk-by-chunk.
  - A run of warm-up matmuls on a scratch tile covers the initial DMA window
    so the PE HAM clock-gate is released (K=8/8) before real work arrives.
  - scores S^T = kT.T @ qT per (head-pair, k-block, 512-chunk); the two heads
    of a pair run concurrently in disjoint 64-row PE groups, one ScalarE exp
    covers both heads' chunks.  Emission is software-pipelined at chunk
    granularity via a filler deque.
  - attn@V: lhsT = v3 with a ones column -> row 64 is the softmax
    denominator; the PSUM drain CAST carries the denominator row along
    (no separate ScalarE copy); reciprocal on DVE + gpsimd broadcast.
  - output projection: half0 (ic 0,1) accumulates early and drains to an
    SBUF staging tile; half1 accumulates ic 2,3 plus an identity-matmul of
    the staged half0 into the same PSUM group, so a single bf16 partial
    [N, D] is written out per core (host adds the 2 per-batch partials).
"""

import numpy as np
from collections import deque

B, N, D = 4, 1024, 1024
H, DH = 16, 64
EPS = 1e-5
P = 128
NHL = 8          # heads per core
FL = NHL * DH    # local features per core (512)
NWARM = 20      # PE warm-up matmuls at kernel start

_cache = {}


def _build_module(has_mask):
    import concourse.bass as bass
    import concourse.bacc as bacc
    import concourse.tile as tile
    import concourse.mybir as mybir
    from concourse.masks import make_identity

    f32 = mybir.dt.float32
    bf16 = mybir.dt.bfloat16
    AF = mybir.ActivationFunctionType
    OP = mybir.AluOpType

    nc = bacc.Bacc("TRN2", target_bir_lowering=False, debug=False, num_devices=8)

    NT = N // P    # 8 token chunks
    ND = D // P    # 8 contraction chunks
    NIC = FL // P  # 4 inner chunks
    NFC = 2 * FL // P  # 8 feature chunks (q/k interleaved: q0,k0,q1,k1,...)

    # wqk packed fc-major: [P, NFC, ND, P]
    xt_in = nc.dram_tensor("xt_in", [P, ND * N], bf16, kind="ExternalInput").ap()
    wqk_in = nc.dram_tensor("wqk_in", [P, NFC * ND * P], bf16, kind="ExternalInput").ap()
    wv_in = nc.dram_tensor("wv_in", [P, ND * FL], bf16, kind="ExternalInput").ap()
    wo_in = nc.dram_tensor("wo_in", [P, NIC * D], bf16, kind="ExternalInput").ap()
    cos_in = nc.dram_tensor("cos_in", [P, N], bf16, kind="ExternalInput").ap()
    sinm_in = nc.dram_tensor("sinm_in", [P, N], bf16, kind="ExternalInput").ap()
    tri_in = nc.dram_tensor("tri_in", [P, P], bf16, kind="ExternalInput").ap()
    perm_in = nc.dram_tensor("perm_in", [P, P], bf16, kind="ExternalInput").ap()
    if has_mask:
        madd_in = nc.dram_tensor("madd_in", [P, 8], f32, kind="ExternalInput").ap()
    out_p = nc.dram_tensor("out_p", [N, D], bf16, kind="ExternalOutput").ap()

    with tile.TileContext(nc) as tc:
        import contextlib
        ctx = contextlib.ExitStack()
        with ctx:
            consts = ctx.enter_context(tc.tile_pool(name="consts", bufs=1))
            big = ctx.enter_context(tc.tile_pool(name="big", bufs=1))
            tmp = ctx.enter_context(tc.tile_pool(name="tmp", bufs=3))
            pt_pool = ctx.enter_context(tc.tile_pool(name="pt_pool", bufs=3))
            avp = ctx.enter_context(tc.tile_pool(name="avp", bufs=3))
            small = ctx.enter_context(tc.tile_pool(name="small", bufs=2))
            bc_pool = ctx.enter_context(tc.tile_pool(name="bc_pool", bufs=3))
            out_pool = ctx.enter_context(tc.tile_pool(name="out_pool", bufs=2))
            psum = ctx.enter_context(tc.tile_pool(name="psum", bufs=2, space="PSUM"))

            # ---- warm-up scratch (device-built, no DMA dependency) ----
            ws = consts.tile([P, 512], bf16)
            nc.vector.memset(ws, 0.125)

            # ---- input DMAs: ONE ordered stream on the sync queue,
            # sequenced in consumption order (first MM needs wqkA + xT dc0)
            wqk_sb = consts.tile([P, NFC, ND, P], bf16)
            nc.sync.dma_start(out=wqk_sb[:, 0:2], in_=wqk_in[:, 0:2 * ND * P])
            xT_sb = big.tile([P, 2, ND, 512], bf16)
            nc.sync.dma_start(out=xT_sb[:, 0], in_=xt_in[:, 0:ND * 512])
            cos_sb = consts.tile([P, N], bf16)
            nc.sync.dma_start(out=cos_sb, in_=cos_in)
            sinm_sb = consts.tile([P, N], bf16)
            nc.sync.dma_start(out=sinm_sb, in_=sinm_in)
            perm_sb = consts.tile([P, P], bf16)
            nc.sync.dma_start(out=perm_sb, in_=perm_in)
            nc.sync.dma_start(out=xT_sb[:, 1], in_=xt_in[:, ND * 512:2 * ND * 512])
            tri_sb = consts.tile([P, P], bf16)
            nc.sync.dma_start(out=tri_sb, in_=tri_in)
            for j in range(1, 4):
                nc.sync.dma_start(
                    out=wqk_sb[:, 2 * j:2 * j + 2],
                    in_=wqk_in[:, 2 * j * ND * P:(2 * j + 2) * ND * P])
            wv_sb = consts.tile([P, ND, FL], bf16)
            nc.sync.dma_start(out=wv_sb, in_=wv_in)
            wo_sb = consts.tile([P, NIC, D], bf16)
            nc.sync.dma_start(out=wo_sb, in_=wo_in)
            if has_mask:
                madd_sb = consts.tile([P, 8], f32)
                nc.sync.dma_start(out=madd_sb, in_=madd_in)

            # ---- constants built on device ----
            idb = consts.tile([P, P], bf16)
            make_identity(nc, idb)

            # ---- PE warm-up: release the HAM clock-gate during the DMA
            # window.  Results are never read; banks recycle via the "s" tag
            # ring before the first real score tile needs them.
            for w in range(NWARM):
                wp = psum.tile([P, 2, 512], f32, tag="s", bufs=2, name=f"warm{w}")
                nc.tensor.matmul(wp[:, 0, :], lhsT=ws[:, 0:P], rhs=ws,
                                 start=True, stop=True, skip_group_check=True)

            # ================= work streams (emitted via filler deque) ======
            rope_sb = big.tile([P, NFC, N], bf16)
            v3 = big.tile([P, NT, NHL, DH + 1], bf16)
            nc.vector.memset(v3[:, :, :, DH:DH + 1], 1.0)
            aoT_sb = big.tile([P, NIC, N], bf16)
            o0_sb = big.tile([P, NT, N], bf16)

            F = deque()          # filler thunks; each emits ~1-2 matmuls
            marks = {}           # group name -> count emitted marker

            def drain(n):
                for _ in range(min(n, len(F))):
                    F.popleft()()

            def drain_group(g):
                while marks.get(g, 0) > 0:
                    F.popleft()()

            def addF(fn, group=None):
                if group is not None:
                    marks[group] = marks.get(group, 0) + 1

                    def wrapped():
                        fn()
                        marks[group] -= 1
                    F.append(wrapped)
                else:
                    F.append(fn)

            # ---- q/k projection + RoPE for one 128-feature chunk ----
            # fci indexes the interleaved order q0,k0,q1,k1,... ; rope_sb
            # rows use the same order.
            def emit_fc(fci, via=None):
                mms = [psum.tile([P, 512], f32, tag="mm", name=f"qk{fci}_{t}")
                       for t in range(2)]

                def rope(t):
                    cs = slice(t * 512, (t + 1) * 512)
                    t1 = tmp.tile([P, 512], bf16, name=f"t1_{fci}_{t}", tag="t1")
                    nc.vector.tensor_tensor(out=t1, in0=mms[t],
                                            in1=cos_sb[:, cs], op=OP.mult)
                    t2 = tmp.tile([P, 512], bf16, name=f"t2_{fci}_{t}", tag="t2")
                    nc.vector.tensor_tensor(out=t2, in0=mms[t],
                                            in1=sinm_sb[:, cs], op=OP.mult)
                    rot = psum.tile([P, 512], f32, tag="av", name=f"rot{fci}_{t}")
                    nc.tensor.matmul(rot, lhsT=perm_sb, rhs=t2,
                                     start=True, stop=True)
                    nc.vector.tensor_tensor(out=rope_sb[:, fci, cs], in0=rot,
                                            in1=t1, op=OP.add)

                def mm_one(t, dc):
                    nc.tensor.matmul(
                        mms[t], lhsT=wqk_sb[:, fci, dc, :],
                        rhs=xT_sb[:, t, dc, :],
                        start=(dc == 0), stop=(dc == ND - 1))

                steps = ([lambda dc=dc: mm_one(0, dc) for dc in range(ND)]
                         + [lambda: rope(0)]
                         + [lambda dc=dc: mm_one(1, dc) for dc in range(ND)]
                         + [lambda: rope(1)])
                if via is None:
                    for s in steps:
                        s()
                else:
                    for s in steps:
                        addF(s, group=via)

            # ---- v projection for one 128-token chunk ----
            def emit_v(kc, via=None):
                mm = psum.tile([P, FL], f32, tag="mm", name=f"v{kc}")

                def vmm(dc):
                    nc.tensor.matmul(
                        mm,
                        lhsT=xT_sb[:, kc // 4, dc, (kc % 4) * P:(kc % 4 + 1) * P],
                        rhs=wv_sb[:, dc, :],
                        start=(dc == 0), stop=(dc == ND - 1))

                def vdrain():
                    nc.scalar.copy(
                        out=v3[:, kc, :, 0:DH],
                        in_=mm.rearrange("p (h c) -> p h c", h=NHL))

                steps = ([lambda dc=dc: vmm(dc) for dc in range(ND)]
                         + [vdrain])
                for s in steps:
                    if via is None:
                        s()
                    else:
                        addF(s, group=via)

            # ---- scores + exp for a head pair, zipped with fillers ----
            all_pt = {}

            def zip_pair(m, fills_per_chunk=3):
                hs = (2 * m, 2 * m + 1)
                qTs = [rope_sb[(h % 2) * 64:(h % 2) * 64 + 64, 2 * m, :]
                       for h in hs]
                kTs = [rope_sb[(h % 2) * 64:(h % 2) * 64 + 64, 2 * m + 1, :]
                       for h in hs]
                pts = []
                for ki in range(NT):
                    q0 = ki * P
                    span = N - q0
                    pt = pt_pool.tile([P, 2, span], bf16, tag=f"pt{ki}",
                                      name=f"pt{m}_{ki}")
                    pts.append(pt)
                    for c in range((span + 511) // 512):
                        cw = min(512, span - c * 512)
                        sp = psum.tile([P, 2, 512], f32, tag="s", bufs=2,
                                       name=f"s{m}_{ki}_{c}")
                        for a in range(2):
                            diag = (c == 0)
                            nc.tensor.matmul(
                                sp[:, a, 0:cw],
                                lhsT=kTs[a][:, ki * P:(ki + 1) * P],
                                rhs=qTs[a][:, q0 + c * 512: q0 + c * 512 + cw],
                                start=True, stop=not diag)
                            if diag:
                                nc.tensor.matmul(
                                    sp[:, a, 0:P], lhsT=tri_sb, rhs=idb,
                                    start=False, stop=True,
                                    skip_group_check=True)
                        if has_mask:
                            nc.scalar.activation(
                                out=pt[:, :, c * 512:c * 512 + cw],
                                in_=sp[:, :, 0:cw], func=AF.Exp,
                                scale=float(DH) ** -0.5,
                                bias=madd_sb[:, ki:ki + 1])
                        else:
                            nc.scalar.activation(
                                out=pt[:, :, c * 512:c * 512 + cw],
                                in_=sp[:, :, 0:cw], func=AF.Exp,
                                scale=float(DH) ** -0.5)
                        drain(fills_per_chunk)
                for a, h in enumerate(hs):
                    all_pt[h] = (pts, a)

            # ---- attn@V + normalize per (head-pair, 512-token chunk) ----
            # The PSUM av tile is freed immediately by one DVE CAST that
            # carries rows 0..64 (values + denominator) into the pair-shared
            # SBUF tile av_pair[65, 1024]; recip + gpsimd broadcast + one DVE
            # multiply per head then normalize straight into aoT.
            av_state = {}

            def emit_av_A(h, cc, via=None):
                pts, a = all_pt[h]
                clo, chi = cc * 512, (cc + 1) * 512
                kis = [ki for ki in range(NT) if ki * P < chi]
                av = psum.tile([DH + 1, 512], f32, tag="av", name=f"av{h}_{cc}")
                pair_key = (h // 2 * 2, cc)
                if a == 0:
                    avs = avp.tile([DH, 1024], f32, name=f"avs{h}_{cc}",
                                   tag="avs")
                    den = small.tile([1, 1024], f32, name=f"den{h}_{cc}",
                                     tag="den")
                    av_state[pair_key] = (avs, den)
                else:
                    avs, den = av_state[pair_key]

                def avmm(idx, ki):
                    qlo = max(clo, ki * P)
                    nc.tensor.matmul(
                        av[:, qlo - clo:512],
                        lhsT=v3[:, ki, h, :],
                        rhs=pts[ki][:, a, qlo - ki * P:chi - ki * P],
                        start=(idx == 0), stop=(idx == len(kis) - 1))

                def drain_ps():
                    eng = nc.scalar if h >= 6 else nc.vector
                    if h >= 6:
                        eng.copy(out=avs[:, a * 512:(a + 1) * 512],
                                 in_=av[0:DH, :])
                    else:
                        eng.tensor_copy(out=avs[:, a * 512:(a + 1) * 512],
                                        in_=av[0:DH, :])
                    # partition 64 -> 0 shift: ScalarE only (DVE is lane-locked)
                    nc.scalar.copy(out=den[0:1, a * 512:(a + 1) * 512],
                                   in_=av[DH:DH + 1, :])

                steps = ([lambda i=i, ki=ki: avmm(i, ki)
                          for i, ki in enumerate(kis)] + [drain_ps])
                for s in steps:
                    if via is None:
                        s()
                    else:
                        addF(s, group=via)

            def emit_av_B(h0, cc, via=None):
                avs, den = av_state[(h0, cc)]
                ic = h0 // 2
                clo, chi = cc * 512, (cc + 1) * 512

                def norm_a(a):
                    rr = small.tile([1, 512], f32, name=f"rr{h0}_{cc}_{a}",
                                    tag="rr")
                    nc.vector.reciprocal_approx_fast(
                        out=rr, in_=den[0:1, a * 512:(a + 1) * 512])
                    bc = bc_pool.tile([DH, 512], f32, name=f"bc{h0}_{cc}_{a}",
                                      tag="bc", bufs=3)
                    nc.gpsimd.partition_broadcast(bc, rr)
                    nc.vector.tensor_tensor(
                        out=aoT_sb[a * DH:(a + 1) * DH, ic, clo:chi],
                        in0=avs[0:DH, a * 512:(a + 1) * 512],
                        in1=bc, op=OP.mult)

                for a in range(2):
                    if via is None:
                        norm_a(a)
                    else:
                        addF(lambda a=a: norm_a(a), group=via)

            def emit_av_pair(h0, h1, via=None, ccs=(0, 1)):
                for cc in ccs:
                    emit_av_A(h0, cc, via=via)
                    emit_av_A(h1, cc, via=via)
                    emit_av_B(h0, cc, via=via)

            def emit_op1_wave(tcis):
                mms_by = {}
                for tci in tcis:
                    tg = "s" if tci % 2 else "mm"
                    mms_by[tci] = [psum.tile([P, 512], f32, tag=tg,
                                             name=f"op1_{tci}_{n2}")
                                   for n2 in range(2)]
                for tci in tcis:
                    for n2 in range(2):
                        nc.tensor.matmul(
                            mms_by[tci][n2],
                            lhsT=aoT_sb[:, 2, tci * P:(tci + 1) * P],
                            rhs=wo_sb[:, 2, n2 * 512:(n2 + 1) * 512],
                            start=True, stop=False)
                        nc.tensor.matmul(
                            mms_by[tci][n2], lhsT=idb,
                            rhs=o0_sb[:, tci, n2 * 512:(n2 + 1) * 512],
                            start=False, stop=False)
                for tci in tcis:
                    for n2 in range(2):
                        nc.tensor.matmul(
                            mms_by[tci][n2],
                            lhsT=aoT_sb[:, 3, tci * P:(tci + 1) * P],
                            rhs=wo_sb[:, 3, n2 * 512:(n2 + 1) * 512],
                            start=False, stop=True)
                for tci in tcis:
                    ot = out_pool.tile([P, N], bf16, name=f"ot{tci}", tag="ot")
                    nc.vector.tensor_copy(out=ot[:, 0:512], in_=mms_by[tci][0])
                    nc.scalar.copy(out=ot[:, 512:1024], in_=mms_by[tci][1])
                    nc.scalar.dma_start(
                        out=out_p[tci * P:(tci + 1) * P, :], in_=ot)

            # ---- output projection ----
            # half 0: accumulate ic 0,1 -> stage to o0_sb (bf16).
            # half 1: accumulate ic 2,3 + idb@o0_sb -> single bf16 output.
            def emit_outproj(half, via=None, tcis=None, alt_ring=False):
                def opmm(ic, n2, tci, mms, last):
                    nc.tensor.matmul(
                        mms[n2], lhsT=aoT_sb[:, ic, tci * P:(tci + 1) * P],
                        rhs=wo_sb[:, ic, n2 * 512:(n2 + 1) * 512],
                        start=(ic == 2 * half), stop=last)

                def opadd0(n2, tci, mms):
                    nc.tensor.matmul(
                        mms[n2], lhsT=idb,
                        rhs=o0_sb[:, tci, n2 * 512:(n2 + 1) * 512],
                        start=False, stop=True)

                def opout(tci, mms):
                    if half == 0:
                        nc.vector.tensor_copy(
                            out=o0_sb[:, tci, 0:512], in_=mms[0])
                        nc.scalar.copy(
                            out=o0_sb[:, tci, 512:1024], in_=mms[1])
                    else:
                        ot = out_pool.tile([P, N], bf16,
                                           name=f"ot{tci}", tag="ot")
                        nc.vector.tensor_copy(out=ot[:, 0:512], in_=mms[0])
                        nc.scalar.copy(out=ot[:, 512:1024], in_=mms[1])
                        nc.scalar.dma_start(
                            out=out_p[tci * P:(tci + 1) * P, :], in_=ot)

                for tci in (range(NT) if tcis is None else tcis):
                    tg = "s" if (alt_ring and tci % 2) else "mm"
                    mms = [psum.tile([P, 512], f32, tag=tg,
                                     name=f"op{half}_{tci}_{n2}")
                           for n2 in range(2)]
                    steps = []
                    for n2 in range(2):
                        for ic in (2 * half, 2 * half + 1):
                            last = (half == 0 and ic == 1)
                            steps.append(
                                lambda ic=ic, n2=n2, t=tci, mm=mms, l=last:
                                opmm(ic, n2, t, mm, l))
                        if half == 1:
                            steps.append(
                                lambda n2=n2, t=tci, mm=mms: opadd0(n2, t, mm))
                    steps.append(lambda t=tci, mm=mms: opout(t, mm))
                    for s in steps:
                        if via is None:
                            s()
                        else:
                            addF(s, group=via)

            # ================= emission schedule ============================
            # NOTE: deferred (via=) steps are strictly FIFO; a consumer may
            # only be emitted (direct or deferred) after every producer it
            # needs is ahead of it in the deque or already drained.
            emit_fc(0)                # q0 (DMA-paced)
            emit_fc(1)                # k0
            emit_fc(2, via="fcB")     # q1
            emit_fc(3, via="fcB")     # k1
            zip_pair(0, fills_per_chunk=3)
            drain_group("fcB")
            emit_fc(4, via="fcC")     # q2
            emit_fc(5, via="fcC")     # k2
            for kc in range(NT):
                emit_v(kc, via="v")
            zip_pair(1, fills_per_chunk=5)
            drain_group("fcC")
            emit_fc(6, via="fcD")     # q3
            emit_fc(7, via="fcD")     # k3
            emit_av_pair(0, 1, via="w1")   # norms after fcD in the deque
            zip_pair(2, fills_per_chunk=6)
            drain_group("fcD")        # flushes v remnants + fcD (FIFO)
            emit_av_pair(2, 3, via="w2")
            emit_outproj(0, via="op0")
            emit_av_pair(4, 5, via="p45")
            zip_pair(3, fills_per_chunk=4)
            while F:
                F.popleft()()
            emit_av_pair(6, 7, ccs=(0,))
            emit_av_A(6, 1)
            emit_av_A(7, 1)
            emit_op1_wave((0, 1))
            emit_av_B(6, 1)
            emit_op1_wave((2, 3))
            emit_op1_wave((4, 5))
            emit_op1_wave((6, 7))

    nc.compile()
    return nc


def kernel(x, rotary_time_emb, x_mask, ln_gamma, ln_beta, w_qkv, w_out, b_out):
    import ml_dtypes
    from concourse import bass_utils

    bf = ml_dtypes.bfloat16
    x = np.asarray(x, np.float32)
    emb = np.asarray(rotary_time_emb, np.float32)
    x_mask = np.asarray(x_mask)
    ln_gamma = np.asarray(ln_gamma, np.float32)
    ln_beta = np.asarray(ln_beta, np.float32)
    w_qkv = np.asarray(w_qkv, np.float32)
    w_out = np.asarray(w_out, np.float32)
    b_out = np.asarray(b_out, np.float32)

    has_mask = bool(np.any(~x_mask.astype(bool)))

    if has_mask not in _cache:
        _cache[has_mask] = _build_module(has_mask)
    nc = _cache[has_mask]

    inner = H * DH
    wq, wk, wv = w_qkv[0:inner], w_qkv[inner:2 * inner], w_qkv[2 * inner:]

    # Host-side LayerNorm (fp32, matches the reference exactly)
    mu = x.mean(-1, keepdims=True)
    var = ((x - mu) ** 2).mean(-1, keepdims=True)
    xn = (x - mu) / np.sqrt(var + EPS) * ln_gamma + ln_beta   # (B, N, D)

    cos = np.cos(emb)                       # (B, N, DH)
    sin = np.sin(emb)

    # block-swap permutation for rotate_half in the transposed layout
    perm = np.zeros((P, P), np.float32)
    o = np.arange(P)
    src = np.where((o % 64) < 32, o + 32, o - 32)
    perm[o, src] = 1.0

    def pack(a):   # [K*P, F] -> [P, K*F] with K-chunks per partition
        kp, f = a.shape
        return np.ascontiguousarray(
            a.reshape(kp // P, P, f).transpose(1, 0, 2).reshape(P, -1)
            .astype(bf))

    in_maps = []
    for core in range(8):
        b, hh = core // 2, core % 2
        sl = slice(hh * FL, (hh + 1) * FL)
        # fc-major q/k interleave: q0,k0,q1,k1,...  each [P, ND, P]
        wq_c, wk_c = wq[sl], wk[sl]          # (FL, D)
        fcs = []
        for i in range(FL // P):
            for w_half in (wq_c, wk_c):
                blk = w_half[i * P:(i + 1) * P]          # (P, D)
                fcs.append(pack(np.ascontiguousarray(blk.T)))  # [P, ND*P]
        wqk_packed = np.ascontiguousarray(
            np.stack(fcs, 1).reshape(P, -1))             # [P, NFC*ND*P]

        xnT = np.ascontiguousarray(xn[b].T)          # (D, N)
        xt_halves = [pack(np.ascontiguousarray(xnT[:, h * 512:(h + 1) * 512]))
                     for h in range(2)]               # each [P, ND*512]
        m = {
            "xt_in": np.ascontiguousarray(np.concatenate(xt_halves, 1)),
            "wqk_in": wqk_packed,
            "wv_in": pack(np.ascontiguousarray(wv[sl].T)),
            "wo_in": pack(np.ascontiguousarray(w_out[:, sl].T)),
            "perm_in": np.ascontiguousarray(perm.astype(bf)),
        }
        cT = cos[b].T                        # (DH, N)
        sT = sin[b].T
        cos2 = np.concatenate([cT, cT], 0)   # (128, N)
        sinm = np.concatenate([sT[32:64], -sT[0:32], sT[32:64], -sT[0:32]], 0)
        m["cos_in"] = np.ascontiguousarray(cos2.astype(bf))
        m["sinm_in"] = np.ascontiguousarray(sinm.astype(bf))
        k_idx = np.arange(P)[:, None]
        q_idx = np.arange(P)[None, :]
        trimask = np.where(k_idx <= q_idx, 0.0, -30000.0)   # [k, q]
        m["tri_in"] = np.ascontiguousarray(trimask.T.astype(bf))
        if has_mask:
            madd = np.where(x_mask[b].astype(bool), 0.0, -30000.0)
            m["madd_in"] = np.ascontiguousarray(
                madd.reshape(8, P).T.astype(np.float32))   # [p, kc]
        in_maps.append(m)

    res = bass_utils.run_bass_kernel_spmd(nc, in_maps, core_ids=list(range(8)))

    out = np.empty((B, N, D), np.float32)
    for b in range(B):
        out[b] = (res.results[2 * b]["out_p"].astype(np.float32)
                  + res.results[2 * b + 1]["out_p"].astype(np.float32))
    out += b_out[None, None, :]
    return out


# revision 16
# speedup vs baseline: 1.1196x; 1.0592x over previous
"""Causal rotary self-attention Trainium2 kernel (8 NeuronCores).

Problem: B=4, N=1024, D=1024, H=16, DH=64.
  LayerNorm -> QKV proj -> RoPE(q,k) -> causal attention -> out proj.

Sharding: 8 cores = 4 batches x 2 head-halves (Megatron-style).  Each core
projects its 8 heads' q/k/v with its weight slice, runs attention for those
heads, and produces a partial output projection in bf16; the host sums the
two partials per batch.

Key design points (v2):
  - LayerNorm is computed entirely on the HOST (xn = (x-mu)*rstd*gamma+beta
    in fp32); the device receives the normalized transposed activations, so
    the device kernel is pure matmul + RoPE + attention.
  - All startup-critical input DMAs are serialized IN CONSUMPTION ORDER on a
    single queue (sync): perm, wqk(q0k0), cos, sinm, xT (dc-major), tri,
    remaining wqk, wv, wo.  Compute chases the DMA stream chunk-by-chunk.
  - A run of warm-up matmuls on a scratch tile covers the initial DMA window
    so the PE HAM clock-gate is released (K=8/8) before real work arrives.
  - scores S^T = kT.T @ qT per (head-pair, k-block, 512-chunk); the two heads
    of a pair run concurrently in disjoint 64-row PE groups, one ScalarE exp
    covers both heads' chunks.  Emission is software-pipelined at chunk
    granularity via a filler deque.
  - attn@V: lhsT = v3 with a ones column -> row 64 is the softmax
    denominator; the PSUM drain CAST carries the denominator row along
    (no separate ScalarE copy); reciprocal on DVE + gpsimd broadcast.
  - output projection: half0 (ic 0,1) accumulates early and drains to an
    SBUF staging tile; half1 accumulates ic 2,3 plus an identity-matmul of
    the staged half0 into the same PSUM group, so a single bf16 partial
    [N, D] is written out per core (host adds the 2 per-batch partials).
"""

import numpy as np
from collections import deque

B, N, D = 4, 1024, 1024
H, DH = 16, 64
EPS = 1e-5
P = 128
NHL = 8          # heads per core
FL = NHL * DH    # local features per core (512)
NWARM = 20      # PE warm-up matmuls at kernel start

_cache = {}


def _build_module(has_mask):
    import concourse.bass as bass
    import concourse.bacc as bacc
    import concourse.tile as tile
    import concourse.mybir as mybir
    from concourse.masks import make_identity

    f32 = mybir.dt.float32
    bf16 = mybir.dt.bfloat16
    AF = mybir.ActivationFunctionType
    OP = mybir.AluOpType

    nc = bacc.Bacc("TRN2", target_bir_lowering=False, debug=False, num_devices=8)

    NT = N // P    # 8 token chunks
    ND = D // P    # 8 contraction chunks
    NIC = FL // P  # 4 inner chunks
    NFC = 2 * FL // P  # 8 feature chunks (q/k interleaved: q0,k0,q1,k1,...)

    # wqk packed fc-major: [P, NFC, ND, P]
    xt_in = nc.dram_tensor("xt_in", [P, ND * N], bf16, kind="ExternalInput").ap()
    wqk_in = nc.dram_tensor("wqk_in", [P, NFC * ND * P], bf16, kind="ExternalInput").ap()
    wv_in = nc.dram_tensor("wv_in", [P, ND * FL], bf16, kind="ExternalInput").ap()
    wo_in = nc.dram_tensor("wo_in", [P, NIC * D], bf16, kind="ExternalInput").ap()
    cos_in = nc.dram_tensor("cos_in", [P, N], bf16, kind="ExternalInput").ap()
    sinm_in = nc.dram_tensor("sinm_in", [P, N], bf16, kind="ExternalInput").ap()
    tri_in = nc.dram_tensor("tri_in", [P, P], bf16, kind="ExternalInput").ap()
    perm_in = nc.dram_tensor("perm_in", [P, P], bf16, kind="ExternalInput").ap()
    if has_mask:
        madd_in = nc.dram_tensor("madd_in", [P, 8], f32, kind="ExternalInput").ap()
    out_p = nc.dram_tensor("out_p", [N, D], bf16, kind="ExternalOutput").ap()

    with tile.TileContext(nc) as tc:
        import contextlib
        ctx = contextlib.ExitStack()
        with ctx:
            consts = ctx.enter_context(tc.tile_pool(name="consts", bufs=1))
            big = ctx.enter_context(tc.tile_pool(name="big", bufs=1))
            tmp = ctx.enter_context(tc.tile_pool(name="tmp", bufs=3))
            pt_pool = ctx.enter_context(tc.tile_pool(name="pt_pool", bufs=3))
            avp = ctx.enter_context(tc.tile_pool(name="avp", bufs=3))
            small = ctx.enter_context(tc.tile_pool(name="small", bufs=2))
            bc_pool = ctx.enter_context(tc.tile_pool(name="bc_pool", bufs=3))
            out_pool = ctx.enter_context(tc.tile_pool(name="out_pool", bufs=2))
            psum = ctx.enter_context(tc.tile_pool(name="psum", bufs=2, space="PSUM"))

            # ---- warm-up scratch (device-built, no DMA dependency) ----
            ws = consts.tile([P, 512], bf16)
            nc.vector.memset(ws, 0.125)

            # ---- input DMAs: ONE ordered stream on the sync queue,
            # sequenced in consumption order (first MM needs wqkA + xT dc0)
            wqk_sb = consts.tile([P, NFC, ND, P], bf16)
            nc.sync.dma_start(out=wqk_sb[:, 0:2], in_=wqk_in[:, 0:2 * ND * P])
            xT_sb = big.tile([P, 2, ND, 512], bf16)
            nc.sync.dma_start(out=xT_sb[:, 0], in_=xt_in[:, 0:ND * 512])
            cos_sb = consts.tile([P, N], bf16)
            nc.sync.dma_start(out=cos_sb, in_=cos_in)
            sinm_sb = consts.tile([P, N], bf16)
            nc.sync.dma_start(out=sinm_sb, in_=sinm_in)
            perm_sb = consts.tile([P, P], bf16)
            nc.sync.dma_start(out=perm_sb, in_=perm_in)
            nc.sync.dma_start(out=xT_sb[:, 1], in_=xt_in[:, ND * 512:2 * ND * 512])
            tri_sb = consts.tile([P, P], bf16)
            nc.sync.dma_start(out=tri_sb, in_=tri_in)
            for j in range(1, 4):
                nc.sync.dma_start(
                    out=wqk_sb[:, 2 * j:2 * j + 2],
                    in_=wqk_in[:, 2 * j * ND * P:(2 * j + 2) * ND * P])
            wv_sb = consts.tile([P, ND, FL], bf16)
            nc.sync.dma_start(out=wv_sb, in_=wv_in)
            wo_sb = consts.tile([P, NIC, D], bf16)
            nc.sync.dma_start(out=wo_sb, in_=wo_in)
            if has_mask:
                madd_sb = consts.tile([P, 8], f32)
                nc.sync.dma_start(out=madd_sb, in_=madd_in)

            # ---- constants built on device ----
            idb = consts.tile([P, P], bf16)
            make_identity(nc, idb)

            # ---- PE warm-up: release the HAM clock-gate during the DMA
            # window.  Results are never read; banks recycle via the "s" tag
            # ring before the first real score tile needs them.
            for w in range(NWARM):
                wp = psum.tile([P, 2, 512], f32, tag="s", bufs=2, name=f"warm{w}")
                nc.tensor.matmul(wp[:, 0, :], lhsT=ws[:, 0:P], rhs=ws,
                                 start=True, stop=True, skip_group_check=True)

            # ================= work streams (emitted via filler deque) ======
            rope_sb = big.tile([P, NFC, N], bf16)
            v3 = big.tile([P, NT, NHL, DH + 1], bf16)
            nc.vector.memset(v3[:, :, :, DH:DH + 1], 1.0)
            aoT_sb = big.tile([P, NIC, N], bf16)
            o0_sb = big.tile([P, NT, N], bf16)

            F = deque()          # filler thunks; each emits ~1-2 matmuls
            marks = {}           # group name -> count emitted marker

            def drain(n):
                for _ in range(min(n, len(F))):
                    F.popleft()()

            def drain_group(g):
                while marks.get(g, 0) > 0:
                    F.popleft()()

            def addF(fn, group=None):
                if group is not None:
                    marks[group] = marks.get(group, 0) + 1

                    def wrapped():
                        fn()
                        marks[group] -= 1
                    F.append(wrapped)
                else:
                    F.append(fn)

            # ---- q/k projection + RoPE for one 128-feature chunk ----
            # fci indexes the interleaved order q0,k0,q1,k1,... ; rope_sb
            # rows use the same order.
            def emit_fc(fci, via=None):
                mms = [psum.tile([P, 512], f32, tag="mm", name=f"qk{fci}_{t}")
                       for t in range(2)]

                def rope(t):
                    cs = slice(t * 512, (t + 1) * 512)
                    t1 = tmp.tile([P, 512], bf16, name=f"t1_{fci}_{t}", tag="t1")
                    nc.vector.tensor_tensor(out=t1, in0=mms[t],
                                            in1=cos_sb[:, cs], op=OP.mult)
                    t2 = tmp.tile([P, 512], bf16, name=f"t2_{fci}_{t}", tag="t2")
                    nc.vector.tensor_tensor(out=t2, in0=mms[t],
                                            in1=sinm_sb[:, cs], op=OP.mult)
                    rot = psum.tile([P, 512], f32, tag="av", name=f"rot{fci}_{t}")
                    nc.tensor.matmul(rot, lhsT=perm_sb, rhs=t2,
                                     start=True, stop=True)
                    nc.vector.tensor_tensor(out=rope_sb[:, fci, cs], in0=rot,
                                            in1=t1, op=OP.add)

                def mm_one(t, dc):
                    nc.tensor.matmul(
                        mms[t], lhsT=wqk_sb[:, fci, dc, :],
                        rhs=xT_sb[:, t, dc, :],
                        start=(dc == 0), stop=(dc == ND - 1))

                steps = ([lambda dc=dc: mm_one(0, dc) for dc in range(ND)]
                         + [lambda: rope(0)]
                         + [lambda dc=dc: mm_one(1, dc) for dc in range(ND)]
                         + [lambda: rope(1)])
                if via is None:
                    for s in steps:
                        s()
                else:
                    for s in steps:
                        addF(s, group=via)

            # ---- v projection for one 128-token chunk ----
            def emit_v(kc, via=None):
                mm = psum.tile([P, FL], f32, tag="mm", name=f"v{kc}")

                def vmm(dc):
                    nc.tensor.matmul(
                        mm,
                        lhsT=xT_sb[:, kc // 4, dc, (kc % 4) * P:(kc % 4 + 1) * P],
                        rhs=wv_sb[:, dc, :],
                        start=(dc == 0), stop=(dc == ND - 1))

                def vdrain():
                    nc.scalar.copy(
                        out=v3[:, kc, :, 0:DH],
                        in_=mm.rearrange("p (h c) -> p h c", h=NHL))

                steps = ([lambda dc=dc: vmm(dc) for dc in range(ND)]
                         + [vdrain])
                for s in steps:
                    if via is None:
                        s()
                    else:
                        addF(s, group=via)

            # ---- scores + exp for a head pair, zipped with fillers ----
            all_pt = {}

            def zip_pair(m, fills_per_chunk=3, heartbeat=False):
                hs = (2 * m, 2 * m + 1)
                qTs = [rope_sb[(h % 2) * 64:(h % 2) * 64 + 64, 2 * m, :]
                       for h in hs]
                kTs = [rope_sb[(h % 2) * 64:(h % 2) * 64 + 64, 2 * m + 1, :]
                       for h in hs]
                pts = []
                for ki in range(NT):
                    q0 = ki * P
                    span = N - q0
                    pt = pt_pool.tile([P, 2, span], bf16, tag=f"pt{ki}",
                                      name=f"pt{m}_{ki}")
                    pts.append(pt)
                    for c in range((span + 511) // 512):
                        cw = min(512, span - c * 512)
                        sp = psum.tile([P, 2, 512], f32, tag="s", bufs=2,
                                       name=f"s{m}_{ki}_{c}")
                        for a in range(2):
                            diag = (c == 0)
                            nc.tensor.matmul(
                                sp[:, a, 0:cw],
                                lhsT=kTs[a][:, ki * P:(ki + 1) * P],
                                rhs=qTs[a][:, q0 + c * 512: q0 + c * 512 + cw],
                                start=True, stop=not diag)
                            if diag:
                                nc.tensor.matmul(
                                    sp[:, a, 0:P], lhsT=tri_sb, rhs=idb,
                                    start=False, stop=True,
                                    skip_group_check=True)
                        if has_mask:
                            nc.scalar.activation(
                                out=pt[:, :, c * 512:c * 512 + cw],
                                in_=sp[:, :, 0:cw], func=AF.Exp,
                                scale=float(DH) ** -0.5,
                                bias=madd_sb[:, ki:ki + 1])
                        else:
                            nc.scalar.activation(
                                out=pt[:, :, c * 512:c * 512 + cw],
                                in_=sp[:, :, 0:cw], func=AF.Exp,
                                scale=float(DH) ** -0.5)
                        drain(fills_per_chunk)
                        if heartbeat:
                            hb = psum.tile([P, 2, 512], f32, tag="s", bufs=2,
                                           name=f"hb{m}_{ki}_{c}")
                            nc.tensor.matmul(hb[:, 0, 0:256], lhsT=ws[:, 0:P],
                                             rhs=ws[:, 0:256], start=True,
                                             stop=True, skip_group_check=True)
                for a, h in enumerate(hs):
                    all_pt[h] = (pts, a)

            # ---- attn@V + normalize per (head-pair, 512-token chunk) ----
            # The PSUM av tile is freed immediately by one DVE CAST that
            # carries rows 0..64 (values + denominator) into the pair-shared
            # SBUF tile av_pair[65, 1024]; recip + gpsimd broadcast + one DVE
            # multiply per head then normalize straight into aoT.
            av_state = {}

            def emit_av_A(h, cc, via=None):
                pts, a = all_pt[h]
                clo, chi = cc * 512, (cc + 1) * 512
                kis = [ki for ki in range(NT) if ki * P < chi]
                av = psum.tile([DH + 1, 512], f32, tag="av", name=f"av{h}_{cc}")
                pair_key = (h // 2 * 2, cc)
                if a == 0:
                    avs = avp.tile([DH, 1024], f32, name=f"avs{h}_{cc}",
                                   tag="avs")
                    den = small.tile([1, 1024], f32, name=f"den{h}_{cc}",
                                     tag="den")
                    av_state[pair_key] = (avs, den)
                else:
                    avs, den = av_state[pair_key]

                def avmm(idx, ki):
                    qlo = max(clo, ki * P)
                    nc.tensor.matmul(
                        av[:, qlo - clo:512],
                        lhsT=v3[:, ki, h, :],
                        rhs=pts[ki][:, a, qlo - ki * P:chi - ki * P],
                        start=(idx == 0), stop=(idx == len(kis) - 1))

                def drain_ps():
                    if h >= 6:
                        nc.scalar.copy(out=avs[:, a * 512:(a + 1) * 512],
                                       in_=av[0:DH, :])
                    else:
                        nc.vector.tensor_copy(
                            out=avs[:, a * 512:(a + 1) * 512], in_=av[0:DH, :])
                    # partition 64 -> 0 shift: standard-op engines only
                    nc.scalar.copy(out=den[0:1, a * 512:(a + 1) * 512],
                                   in_=av[DH:DH + 1, :])

                steps = ([lambda i=i, ki=ki: avmm(i, ki)
                          for i, ki in enumerate(kis)] + [drain_ps])
                for s in steps:
                    if via is None:
                        s()
                    else:
                        addF(s, group=via)

            def emit_av_B(h0, cc, via=None):
                avs, den = av_state[(h0, cc)]
                ic = h0 // 2
                clo, chi = cc * 512, (cc + 1) * 512

                def norm_a(a):
                    rr = small.tile([1, 512], f32, name=f"rr{h0}_{cc}_{a}",
                                    tag="rr")
                    nc.vector.reciprocal_approx_fast(
                        out=rr, in_=den[0:1, a * 512:(a + 1) * 512])
                    bc = bc_pool.tile([DH, 512], f32, name=f"bc{h0}_{cc}_{a}",
                                      tag="bc", bufs=3)
                    nc.gpsimd.partition_broadcast(bc, rr)
                    nc.vector.tensor_tensor(
                        out=aoT_sb[a * DH:(a + 1) * DH, ic, clo:chi],
                        in0=avs[0:DH, a * 512:(a + 1) * 512],
                        in1=bc, op=OP.mult)

                for a in range(2):
                    if via is None:
                        norm_a(a)
                    else:
                        addF(lambda a=a: norm_a(a), group=via)

            def emit_av_pair(h0, h1, via=None, ccs=(0, 1)):
                for cc in ccs:
                    emit_av_A(h0, cc, via=via)
                    emit_av_A(h1, cc, via=via)
                    emit_av_B(h0, cc, via=via)

            def emit_op1_wave(tcis):
                mms_by = {}
                for tci in tcis:
                    tg = "s" if tci % 2 else "mm"
                    mms_by[tci] = [psum.tile([P, 512], f32, tag=tg,
                                             name=f"op1_{tci}_{n2}")
                                   for n2 in range(2)]
                for tci in tcis:
                    for n2 in range(2):
                        nc.tensor.matmul(
                            mms_by[tci][n2],
                            lhsT=aoT_sb[:, 2, tci * P:(tci + 1) * P],
                            rhs=wo_sb[:, 2, n2 * 512:(n2 + 1) * 512],
                            start=True, stop=False)
                        nc.tensor.matmul(
                            mms_by[tci][n2], lhsT=idb,
                            rhs=o0_sb[:, tci, n2 * 512:(n2 + 1) * 512],
                            start=False, stop=False)
                for tci in tcis:
                    for n2 in range(2):
                        nc.tensor.matmul(
                            mms_by[tci][n2],
                            lhsT=aoT_sb[:, 3, tci * P:(tci + 1) * P],
                            rhs=wo_sb[:, 3, n2 * 512:(n2 + 1) * 512],
                            start=False, stop=True)
                for tci in tcis:
                    ot = out_pool.tile([P, N], bf16, name=f"ot{tci}", tag="ot")
                    nc.vector.tensor_copy(out=ot[:, 0:512], in_=mms_by[tci][0])
                    nc.scalar.copy(out=ot[:, 512:1024], in_=mms_by[tci][1])
                    nc.scalar.dma_start(
                        out=out_p[tci * P:(tci + 1) * P, :], in_=ot)

            # ---- output projection ----
            # half 0: accumulate ic 0,1 -> stage to o0_sb (bf16).
            # half 1: accumulate ic 2,3 + idb@o0_sb -> single bf16 output.
            def emit_outproj(half, via=None, tcis=None, alt_ring=False):
                def opmm(ic, n2, tci, mms, last):
                    nc.tensor.matmul(
                        mms[n2], lhsT=aoT_sb[:, ic, tci * P:(tci + 1) * P],
                        rhs=wo_sb[:, ic, n2 * 512:(n2 + 1) * 512],
                        start=(ic == 2 * half), stop=last)

                def opadd0(n2, tci, mms):
                    nc.tensor.matmul(
                        mms[n2], lhsT=idb,
                        rhs=o0_sb[:, tci, n2 * 512:(n2 + 1) * 512],
                        start=False, stop=True)

                def opout(tci, mms):
                    if half == 0:
                        nc.vector.tensor_copy(
                            out=o0_sb[:, tci, 0:512], in_=mms[0])
                        nc.scalar.copy(
                            out=o0_sb[:, tci, 512:1024], in_=mms[1])
                    else:
                        ot = out_pool.tile([P, N], bf16,
                                           name=f"ot{tci}", tag="ot")
                        nc.vector.tensor_copy(out=ot[:, 0:512], in_=mms[0])
                        nc.scalar.copy(out=ot[:, 512:1024], in_=mms[1])
                        nc.scalar.dma_start(
                            out=out_p[tci * P:(tci + 1) * P, :], in_=ot)

                for tci in (range(NT) if tcis is None else tcis):
                    tg = "s" if (alt_ring and tci % 2) else "mm"
                    mms = [psum.tile([P, 512], f32, tag=tg,
                                     name=f"op{half}_{tci}_{n2}")
                           for n2 in range(2)]
                    steps = []
                    for n2 in range(2):
                        for ic in (2 * half, 2 * half + 1):
                            last = (half == 0 and ic == 1)
                            steps.append(
                                lambda ic=ic, n2=n2, t=tci, mm=mms, l=last:
                                opmm(ic, n2, t, mm, l))
                        if half == 1:
                            steps.append(
                                lambda n2=n2, t=tci, mm=mms: opadd0(n2, t, mm))
                    steps.append(lambda t=tci, mm=mms: opout(t, mm))
                    for s in steps:
                        if via is None:
                            s()
                        else:
                            addF(s, group=via)

            # ================= emission schedule ============================
            # NOTE: deferred (via=) steps are strictly FIFO; a consumer may
            # only be emitted (direct or deferred) after every producer it
            # needs is ahead of it in the deque or already drained.
            emit_fc(0)                # q0 (DMA-paced)
            emit_fc(1)                # k0
            emit_fc(2, via="fcB")     # q1
            emit_fc(3, via="fcB")     # k1
            zip_pair(0, fills_per_chunk=3)
            drain_group("fcB")
            emit_fc(4, via="fcC")     # q2
            emit_fc(5, via="fcC")     # k2
            for kc in range(NT):
                emit_v(kc, via="v")
            zip_pair(1, fills_per_chunk=6)
            drain_group("fcC")
            emit_fc(6, via="fcD")     # q3
            emit_fc(7, via="fcD")     # k3
            emit_av_pair(0, 1, via="w1")   # norms after fcD in the deque
            zip_pair(2, fills_per_chunk=7, heartbeat=True)
            drain_group("fcD")        # flushes v remnants + fcD (FIFO)
            emit_av_pair(2, 3, via="w2")
            emit_outproj(0, via="op0")
            emit_av_pair(4, 5, via="p45")
            zip_pair(3, fills_per_chunk=6, heartbeat=True)
            while F:
                F.popleft()()
            emit_av_pair(6, 7, ccs=(0,))
            emit_av_A(6, 1)
            emit_av_A(7, 1)
            emit_op1_wave((0, 1))
            emit_av_B(6, 1)
            emit_op1_wave((2, 3))
            emit_op1_wave((4, 5))
            emit_op1_wave((6, 7))

    nc.compile()
    return nc


def kernel(x, rotary_time_emb, x_mask, ln_gamma, ln_beta, w_qkv, w_out, b_out):
    import ml_dtypes
    from concourse import bass_utils

    bf = ml_dtypes.bfloat16
    x = np.asarray(x, np.float32)
    emb = np.asarray(rotary_time_emb, np.float32)
    x_mask = np.asarray(x_mask)
    ln_gamma = np.asarray(ln_gamma, np.float32)
    ln_beta = np.asarray(ln_beta, np.float32)
    w_qkv = np.asarray(w_qkv, np.float32)
    w_out = np.asarray(w_out, np.float32)
    b_out = np.asarray(b_out, np.float32)

    has_mask = bool(np.any(~x_mask.astype(bool)))

    if has_mask not in _cache:
        _cache[has_mask] = _build_module(has_mask)
    nc = _cache[has_mask]

    inner = H * DH
    wq, wk, wv = w_qkv[0:inner], w_qkv[inner:2 * inner], w_qkv[2 * inner:]

    # Host-side LayerNorm (fp32, matches the reference exactly)
    mu = x.mean(-1, keepdims=True)
    var = ((x - mu) ** 2).mean(-1, keepdims=True)
    xn = (x - mu) / np.sqrt(var + EPS) * ln_gamma + ln_beta   # (B, N, D)

    cos = np.cos(emb)                       # (B, N, DH)
    sin = np.sin(emb)

    # block-swap permutation for rotate_half in the transposed layout
    perm = np.zeros((P, P), np.float32)
    o = np.arange(P)
    src = np.where((o % 64) < 32, o + 32, o - 32)
    perm[o, src] = 1.0

    def pack(a):   # [K*P, F] -> [P, K*F] with K-chunks per partition
        kp, f = a.shape
        return np.ascontiguousarray(
            a.reshape(kp // P, P, f).transpose(1, 0, 2).reshape(P, -1)
            .astype(bf))

    in_maps = []
    for core in range(8):
        b, hh = core // 2, core % 2
        sl = slice(hh * FL, (hh + 1) * FL)
        # fc-major q/k interleave: q0,k0,q1,k1,...  each [P, ND, P]
        wq_c, wk_c = wq[sl], wk[sl]          # (FL, D)
        fcs = []
        for i in range(FL // P):
            for w_half in (wq_c, wk_c):
                blk = w_half[i * P:(i + 1) * P]          # (P, D)
                fcs.append(pack(np.ascontiguousarray(blk.T)))  # [P, ND*P]
        wqk_packed = np.ascontiguousarray(
            np.stack(fcs, 1).reshape(P, -1))             # [P, NFC*ND*P]

        xnT = np.ascontiguousarray(xn[b].T)          # (D, N)
        xt_halves = [pack(np.ascontiguousarray(xnT[:, h * 512:(h + 1) * 512]))
                     for h in range(2)]               # each [P, ND*512]
        m = {
            "xt_in": np.ascontiguousarray(np.concatenate(xt_halves, 1)),
            "wqk_in": wqk_packed,
            "wv_in": pack(np.ascontiguousarray(wv[sl].T)),
            "wo_in": pack(np.ascontiguousarray(w_out[:, sl].T)),
            "perm_in": np.ascontiguousarray(perm.astype(bf)),
        }
        cT = cos[b].T                        # (DH, N)
        sT = sin[b].T
        cos2 = np.concatenate([cT, cT], 0)   # (128, N)
        sinm = np.concatenate([sT[32:64], -sT[0:32], sT[32:64], -sT[0:32]], 0)
        m["cos_in"] = np.ascontiguousarray(cos2.astype(bf))
        m["sinm_in"] = np.ascontiguousarray(sinm.astype(bf))
        k_idx = np.arange(P)[:, None]
        q_idx = np.arange(P)[None, :]
        trimask = np.where(k_idx <= q_idx, 0.0, -30000.0)   # [k, q]
        m["tri_in"] = np.ascontiguousarray(trimask.T.astype(bf))
        if has_mask:
            madd = np.where(x_mask[b].astype(bool), 0.0, -30000.0)
            m["madd_in"] = np.ascontiguousarray(
                madd.reshape(8, P).T.astype(np.float32))   # [p, kc]
        in_maps.append(m)

    res = bass_utils.run_bass_kernel_spmd(nc, in_maps, core_ids=list(range(8)))

    out = np.empty((B, N, D), np.float32)
    for b in range(B):
        out[b] = (res.results[2 * b]["out_p"].astype(np.float32)
                  + res.results[2 * b + 1]["out_p"].astype(np.float32))
    out += b_out[None, None, :]
    return out
